# revision 4
# baseline (speedup 1.0000x reference)
"""Fused multi-head attention (RoPE + GQA + softmax + o_proj) on 8 Trainium2 cores.

Sharding v2: core c handles batch b = c//2 and head-group hg = c%2
(8 q-heads / 2 kv-heads), ALL 2048 queries.  Each core computes K/V for
only its kv heads, attention for its 8 q heads, and a PARTIAL o_proj
(contracted over its heads).  The host sums the two partial outputs per
batch (the "all-reduce after o_proj" of the tensor-parallel sharding).

This removes the K/V-projection duplication of the (batch, query-half)
sharding: per-core matmul work is exactly 1/8 of the model total.

Everything runs in bf16 (1 cycle/row on the PE, same as f32r, but 2x on
DVE and half the DMA/SBUF), accumulating in f32 PSUM.

Per-core layouts (partition dim first):
  xT  [128, 16, S]   x[b]^T swizzled: partition=d%128, (dchunk, s)  bf16
  kt  [128, 2, S]    roped K, partition=d%128 of the kv head        bf16
  vt  [128, 16, 256] V, partition=s%128, (schunk, j of 2 kv heads)  bf16
  qh  [8][128, S]    roped Q per head, partition=d%128              bf16
  att [8][128, 512]  per q-tile: attention out, partition=j         bf16
"""

import sys

sys.path.insert(0, "/opt/trn_rl_repo")

import math

import numpy as np
import ml_dtypes

import concourse.bass as bass
import concourse.mybir as mybir
import concourse.tile as tile
from concourse import bacc
from concourse.bass_utils import run_bass_kernel_spmd

P = 128
B, S, HID = 4, 2048, 2048
H, HKV, D = 16, 4, 128
HL = H // 2  # 8 q heads per core
KVL = HKV // 2  # 2 kv heads per core
DC = HID // P  # 16
KVJ = KVL * D  # 256
ST = 512  # s-tile for projections; also q-tile for attention
NST = S // ST  # 4
NKC = S // P  # 16 key chunks
ROPE_THETA = 10000.0

F32 = mybir.dt.float32
BF16 = mybir.dt.bfloat16
AL = mybir.AluOpType
AF = mybir.ActivationFunctionType

_CACHE = {}


def build_nc():
    if "nc" in _CACHE:
        return _CACHE["nc"]
    nc = bacc.Bacc("TRN2", target_bir_lowering=False)

    xT = nc.dram_tensor("xT", (P, DC, S), BF16, kind="ExternalInput")
    wq = nc.dram_tensor("wq", (HL, P, DC, P), BF16, kind="ExternalInput")
    wk = nc.dram_tensor("wk", (P, DC, KVJ), BF16, kind="ExternalInput")
    wv = nc.dram_tensor("wv", (P, DC, KVJ), BF16, kind="ExternalInput")
    wo = nc.dram_tensor("wo", (HL, P, HID), BF16, kind="ExternalInput")
    cos_q = nc.dram_tensor("cos_q", (P, S), BF16, kind="ExternalInput")
    sin_q = nc.dram_tensor("sin_q", (P, S), BF16, kind="ExternalInput")
    cos_k = nc.dram_tensor("cos_k", (P, S), BF16, kind="ExternalInput")
    sin_k = nc.dram_tensor("sin_k", (P, S), BF16, kind="ExternalInput")
    pmat = nc.dram_tensor("pmat", (P, P), BF16, kind="ExternalInput")
    ones = nc.dram_tensor("ones", (P, 1), BF16, kind="ExternalInput")
    out = nc.dram_tensor("out", (S, HID), F32, kind="ExternalOutput")

    with tile.TileContext(nc) as tc:
        with (
            tc.tile_pool(name="consts", bufs=1) as consts,
            tc.tile_pool(name="kt", bufs=1) as ktp,
            tc.tile_pool(name="vt", bufs=1) as vtp,
            tc.tile_pool(name="qh", bufs=1) as qhp,
            tc.tile_pool(name="wop", bufs=1) as wop,
        ):
            pm_t = consts.tile([P, P], BF16)
            nc.sync.dma_start(pm_t[:], pmat.ap())
            ones_t = consts.tile([P, 1], BF16)
            nc.sync.dma_start(ones_t[:], ones.ap())
            kt = ktp.tile([P, KVL, S], BF16)
            vt = vtp.tile([P, NKC, KVJ], BF16)
            qh = [qhp.tile([P, S], BF16, name=f"qh{h}") for h in range(HL)]
            wo_sb = [wop.tile([P, HID], BF16, name=f"wo{h}") for h in range(HL)]

            def rope(raw_ps, swp_ps, cos_sl, sin_sl, dst, work, w, tagp):
                # dst = raw*cos + (Pmat @ raw)*sin ; PSUM->SBUF copies on ACT,
                # bf16 multiplies/add on DVE (2x mode).
                raw_sb = work.tile([P, w], BF16, tag=f"{tagp}raw", name="rp_raw")
                nc.scalar.copy(raw_sb[:], raw_ps)
                nc.tensor.matmul(swp_ps, lhsT=pm_t[:], rhs=raw_sb[:], start=True, stop=True)
                swp_sb = work.tile([P, w], BF16, tag=f"{tagp}swp", name="rp_swp")
                nc.scalar.copy(swp_sb[:], swp_ps)
                ta = work.tile([P, w], BF16, tag=f"{tagp}a", name="rp_a")
                nc.vector.tensor_tensor(ta[:], raw_sb[:], cos_sl, AL.mult)
                tb = work.tile([P, w], BF16, tag=f"{tagp}b", name="rp_b")
                nc.vector.tensor_tensor(tb[:], swp_sb[:], sin_sl, AL.mult)
                nc.vector.tensor_tensor(dst, ta[:], tb[:], AL.add)

            # ---- Phase KV: K^T (roped) and V, streaming xT tiles ----
            with (
                tc.tile_pool(name="xin", bufs=2) as xin,
                tc.tile_pool(name="ktab", bufs=1) as ktab,
                tc.tile_pool(name="wkp", bufs=1) as wkp,
                tc.tile_pool(name="ropeK", bufs=2) as ropeK,
                tc.tile_pool(name="ppP", bufs=2, space="PSUM") as ppP,
                tc.tile_pool(name="ppS", bufs=2, space="PSUM") as ppS,
                tc.tile_pool(name="ppV", bufs=2, space="PSUM") as ppV,
            ):
                ck_t = ktab.tile([P, S], BF16, name="cosk")
                nc.sync.dma_start(ck_t[:], cos_k.ap())
                sk_t = ktab.tile([P, S], BF16, name="sink")
                nc.sync.dma_start(sk_t[:], sin_k.ap())
                wk_sb = wkp.tile([P, DC, KVJ], BF16, name="wk")
                wv_sb = wkp.tile([P, DC, KVJ], BF16, name="wv")
                xt0 = xin.tile([P, DC, ST], BF16, tag="xt", name="xt0")
                nc.sync.dma_start(xt0[:], xT.ap()[:, :, 0:ST])
                nc.sync.dma_start(wk_sb[:], wk.ap())
                nc.sync.dma_start(wv_sb[:], wv.ap())
                for st in range(NST):
                    if st == 0:
                        xt = xt0
                    else:
                        xt = xin.tile([P, DC, ST], BF16, tag="xt", name=f"xt{st}")
                        nc.sync.dma_start(xt[:], xT.ap()[:, :, st * ST:(st + 1) * ST])
                    sl = slice(st * ST, (st + 1) * ST)
                    for jc in range(KVL):
                        pk = ppP.tile([P, ST], F32, tag="pk")
                        for dc in range(DC):
                            nc.tensor.matmul(
                                pk[:],
                                lhsT=wk_sb[:, dc, jc * P:(jc + 1) * P],
                                rhs=xt[:, dc, :],
                                start=(dc == 0),
                                stop=(dc == DC - 1),
                            )
                        swp = ppS.tile([P, ST], F32, tag="swp")
                        rope(pk[:], swp[:], ck_t[:, sl], sk_t[:, sl],
                             kt[:, jc, sl], ropeK, ST, "k")
                    pv = ppV.tile([P, NST, KVJ], F32, tag="pv")
                    for si in range(NST):
                        for dc in range(DC):
                            nc.tensor.matmul(
                                pv[:, si, :],
                                lhsT=xt[:, dc, si * P:(si + 1) * P],
                                rhs=wv_sb[:, dc, :],
                                start=(dc == 0),
                                stop=(dc == DC - 1),
                            )
                    nc.scalar.copy(vt[:, st * NST:(st + 1) * NST, :], pv[:])

            # ---- Phase Q: all 8 heads, roped ----
            with (
                tc.tile_pool(name="xinq", bufs=2) as xinq,
                tc.tile_pool(name="qtab", bufs=1) as qtab,
                tc.tile_pool(name="wqp", bufs=1) as wqp,
                tc.tile_pool(name="ropeQ", bufs=2) as ropeQ,
                tc.tile_pool(name="ppQ", bufs=2, space="PSUM") as ppQ,
                tc.tile_pool(name="ppSQ", bufs=2, space="PSUM") as ppSQ,
            ):
                cq_t = qtab.tile([P, S], BF16, name="cosq")
                nc.sync.dma_start(cq_t[:], cos_q.ap())
                sq_t = qtab.tile([P, S], BF16, name="sinq")
                nc.sync.dma_start(sq_t[:], sin_q.ap())
                wq_sb = [wqp.tile([P, DC, P], BF16, name=f"wq{h}") for h in range(HL)]
                xtq0 = xinq.tile([P, DC, ST], BF16, tag="xtq", name="xtq0")
                nc.sync.dma_start(xtq0[:], xT.ap()[:, :, 0:ST])
                for h in range(HL):
                    nc.sync.dma_start(wq_sb[h][:], wq.ap()[h])
                # prefetch wo for phase B while xT streams
                for h in range(HL):
                    nc.sync.dma_start(wo_sb[h][:], wo.ap()[h])
                for st in range(NST):
                    if st == 0:
                        xt = xtq0
                    else:
                        xt = xinq.tile([P, DC, ST], BF16, tag="xtq", name=f"xtq{st}")
                        nc.sync.dma_start(xt[:], xT.ap()[:, :, st * ST:(st + 1) * ST])
                    sl = slice(st * ST, (st + 1) * ST)
                    for h in range(HL):
                        pq = ppQ.tile([P, ST], F32, tag="pq")
                        for dc in range(DC):
                            nc.tensor.matmul(
                                pq[:],
                                lhsT=wq_sb[h][:, dc, :],
                                rhs=xt[:, dc, :],
                                start=(dc == 0),
                                stop=(dc == DC - 1),
                            )
                        swp = ppSQ.tile([P, ST], F32, tag="swq")
                        rope(pq[:], swp[:], cq_t[:, sl], sq_t[:, sl],
                             qh[h][:, sl], ropeQ, ST, "q")

            # ---- Phase B: attention + fused o_proj per q-tile ----
            with (
                tc.tile_pool(name="ptp", bufs=2) as ptp,
                tc.tile_pool(name="tree", bufs=2) as treep,
                tc.tile_pool(name="attp", bufs=1) as attp,
                tc.tile_pool(name="nrm", bufs=2) as nrmp,
                tc.tile_pool(name="outp", bufs=3) as outp,
                tc.tile_pool(name="ppSc", bufs=2, space="PSUM") as ppSc,
                tc.tile_pool(name="ppAv", bufs=1, space="PSUM") as ppAv,
                tc.tile_pool(name="ppDn", bufs=1, space="PSUM") as ppDn,
                tc.tile_pool(name="ppO", bufs=2, space="PSUM") as ppO,
            ):
                att_sb = [attp.tile([P, ST], BF16, name=f"att{h}") for h in range(HL)]
                for qt in range(NST):
                    qsl = slice(qt * ST, (qt + 1) * ST)
                    for h in range(HL):
                        kv = h // (HL // KVL)
                        av = ppAv.tile([P, ST], F32, tag="av")
                        den = ppDn.tile([1, ST], F32, tag="den")
                        pt = [None] * 8
                        for kp in range(8):
                            sc_ps = ppSc.tile([P, 2 * ST], F32, tag="scores")
                            for i in range(2):
                                kc = kp * 2 + i
                                nc.tensor.matmul(
                                    sc_ps[:, i * ST:(i + 1) * ST],
                                    lhsT=kt[:, kv, kc * P:(kc + 1) * P],
                                    rhs=qh[h][:, qsl],
                                    start=True,
                                    stop=True,
                                )
                            pt[kp] = ptp.tile([P, 2 * ST], BF16, tag=f"pt{kp}",
                                              name=f"pt{kp}")
                            nc.scalar.activation(pt[kp][:], sc_ps[:], AF.Exp)
                            for i in range(2):
                                kc = kp * 2 + i
                                nc.tensor.matmul(
                                    av[:],
                                    lhsT=vt[:, kc, kv * P:(kv + 1) * P],
                                    rhs=pt[kp][:, i * ST:(i + 1) * ST],
                                    start=(kc == 0),
                                    stop=(kc == NKC - 1),
                                )
                        # sum tree for the denominator: L1 on Pool, rest on DVE
                        t1 = [treep.tile([P, 2 * ST], BF16, tag=f"t1{j}", name=f"t1{j}")
                              for j in range(4)]
                        for j in range(4):
                            nc.gpsimd.tensor_tensor(
                                t1[j][:], pt[2 * j][:], pt[2 * j + 1][:], AL.add
                            )
                        t2 = [treep.tile([P, 2 * ST], BF16, tag=f"t2{j}", name=f"t2{j}")
                              for j in range(2)]
                        for j in range(2):
                            nc.vector.tensor_tensor(
                                t2[j][:], t1[2 * j][:], t1[2 * j + 1][:], AL.add
                            )
                        t3 = treep.tile([P, 2 * ST], BF16, tag="t3", name="t3")
                        nc.vector.tensor_tensor(t3[:], t2[0][:], t2[1][:], AL.add)
                        t4 = treep.tile([P, ST], BF16, tag="t4", name="t4")
                        nc.vector.tensor_tensor(
                            t4[:], t3[:, 0:ST], t3[:, ST:2 * ST], AL.add
                        )
                        nc.tensor.matmul(
                            den[:], lhsT=ones_t[:], rhs=t4[:], start=True, stop=True
                        )
                        r_row = nrmp.tile([1, ST], F32, tag="rrow", name="rrow")
                        nc.vector.reciprocal(r_row[:], den[:])
                        rb = nrmp.tile([P, ST], F32, tag="rb", name="rb")
                        nc.gpsimd.partition_broadcast(rb[:], r_row[:])
                        nc.vector.tensor_tensor(att_sb[h][:], av[:], rb[:], AL.mult)
                    # o_proj for this q-tile (partial over this core's heads)
                    for qc in range(ST // P):
                        for ot in range(HID // ST):
                            po = ppO.tile([P, ST], F32, tag="po")
                            for h in range(HL):
                                nc.tensor.matmul(
                                    po[:],
                                    lhsT=att_sb[h][:, qc * P:(qc + 1) * P],
                                    rhs=wo_sb[h][:, ot * ST:(ot + 1) * ST],
                                    start=(h == 0),
                                    stop=(h == HL - 1),
                                )
                            out_t = outp.tile([P, ST], F32, tag="outt")
                            nc.vector.tensor_copy(out_t[:], po[:])
                            nc.sync.dma_start(
                                out.ap()[qt * ST + qc * P:qt * ST + (qc + 1) * P,
                                         ot * ST:(ot + 1) * ST],
                                out_t[:],
                            )

    nc.compile()
    _CACHE["nc"] = nc
    return nc


def _host_inputs(x, Wq, Wk, Wv, Wo):
    """Build the 8 per-core input maps (numpy only)."""
    bf = ml_dtypes.bfloat16

    # rope tables: row j uses frequency j%64
    inv_ts = ROPE_THETA ** (-2.0 * np.arange(D // 2) / D)
    inv_full = np.concatenate([inv_ts, inv_ts])
    pos = np.arange(S, dtype=np.float64)
    ang = inv_full[:, None] * pos[None, :]
    cos_k = np.cos(ang).astype(bf)
    sin_k = np.sin(ang).astype(bf)
    scale = 1.0 / math.sqrt(D)
    cos_q = (np.cos(ang) * scale).astype(bf)
    sin_q = (np.sin(ang) * scale).astype(bf)

    pmat = np.zeros((P, P), np.float32)  # lhsT: swap[i] = -q[i+64] (i<64), +q[i-64]
    for i in range(64):
        pmat[i + 64, i] = -1.0
        pmat[i, i + 64] = 1.0
    pmat = pmat.astype(bf)
    ones = np.ones((P, 1), bf)

    in_maps = []
    for c in range(8):
        b, hg = c // 2, c % 2
        hsl = slice(hg * HL, (hg + 1) * HL)
        kvsl = slice(hg * KVL, (hg + 1) * KVL)
        xTb = np.ascontiguousarray(
            x[b].T.reshape(DC, P, S).transpose(1, 0, 2)
        ).astype(bf)  # [p, dc, s]
        wq_sw = np.ascontiguousarray(
            Wq[:, hsl, :].reshape(DC, P, HL, P).transpose(2, 1, 0, 3)
        ).astype(bf)  # [h, p, dc, j]
        wk_sw = np.ascontiguousarray(
            Wk[:, kvsl, :].reshape(DC, P, KVJ).transpose(1, 0, 2)
        ).astype(bf)  # [p, dc, j]
        wv_sw = np.ascontiguousarray(
            Wv[:, kvsl, :].reshape(DC, P, KVJ).transpose(1, 0, 2)
        ).astype(bf)
        wo_sw = np.ascontiguousarray(Wo[hsl]).astype(bf)  # [h, j(=d), o]
        in_maps.append(
            {
                "xT": xTb,
                "wq": wq_sw,
                "wk": wk_sw,
                "wv": wv_sw,
                "wo": wo_sw,
                "cos_q": cos_q,
                "sin_q": sin_q,
                "cos_k": cos_k,
                "sin_k": sin_k,
                "pmat": pmat,
                "ones": ones,
            }
        )
    return in_maps


def kernel(x, Wq, Wk, Wv, Wo, _trace=False):
    x, Wq, Wk, Wv, Wo = (np.asarray(a, dtype=np.float32) for a in (x, Wq, Wk, Wv, Wo))
    nc = build_nc()
    in_maps = _host_inputs(x, Wq, Wk, Wv, Wo)
    res = run_bass_kernel_spmd(nc, in_maps, core_ids=list(range(8)), trace=_trace)
    out = np.empty((B, S, HID), np.float32)
    for b in range(B):
        out[b] = res.results[2 * b]["out"]
        out[b] += res.results[2 * b + 1]["out"]
    if _trace:
        kernel.last_results = res
    return out


# revision 6
# speedup vs baseline: 1.1368x; 1.1368x over previous
"""Fused multi-head attention (RoPE + GQA + softmax + o_proj) on 8 Trainium2 cores.

Sharding: core c handles batch b = c//2 and head-group hg = c%2
(8 q-heads / 2 kv-heads), ALL 2048 queries.  Each core computes K/V for
only its kv heads, attention for its 8 q heads, and a PARTIAL o_proj
(contracted over its heads).  The host sums the two partial outputs per
batch (the "all-reduce after o_proj" of the tensor-parallel sharding).
Per-core matmul work is exactly 1/8 of the model total.

Everything runs in bf16 (1 cycle/row on the PE, same as f32r, but 2x on
DVE and half the DMA/SBUF), accumulating in f32 PSUM.

Pipelining (PE program order is execution order per engine):
 - rope's swap matmul for iteration u is emitted inside iteration u+1 so
   the PE never waits on the ACT psum->sbuf copy.
 - attention unit u = (qt, h): av matmuls trail the score matmuls by 2
   kp-steps so the ACT exp pipeline stays ahead of the PE.
 - the denominator matmul + normalize of unit u are emitted inside unit
   u+1 (tree latency hidden); den borrows a scores-ring PSUM slot.
 - o_proj of q-tile qt is emitted inside unit (qt+1, h0) so ACT/DVE of
   the next tile's units overlap its matmuls.

Per-core layouts (partition dim first):
  xT  [128, 16, S]   x[b]^T swizzled: partition=d%128, (dchunk, s)  bf16
  kt  [128, 2, S]    roped K, partition=d%128 of the kv head        bf16
  vt  [128, 16, 256] V, partition=s%128, (schunk, j of 2 kv heads)  bf16
  qh  [8][128, S]    roped Q per head, partition=d%128              bf16
  att [2][8][128, 512]  per q-tile: attention out, partition=j      bf16
"""

import sys

sys.path.insert(0, "/opt/trn_rl_repo")

import math

import numpy as np
import ml_dtypes

import concourse.bass as bass
import concourse.mybir as mybir
import concourse.tile as tile
from concourse import bacc
from concourse.bass_utils import run_bass_kernel_spmd

P = 128
B, S, HID = 4, 2048, 2048
H, HKV, D = 16, 4, 128
HL = H // 2  # 8 q heads per core
KVL = HKV // 2  # 2 kv heads per core
DC = HID // P  # 16
KVJ = KVL * D  # 256
ST = 512  # s-tile for projections; also q-tile for attention
NST = S // ST  # 4
NKC = S // P  # 16 key chunks
ROPE_THETA = 10000.0

F32 = mybir.dt.float32
BF16 = mybir.dt.bfloat16
AL = mybir.AluOpType
AF = mybir.ActivationFunctionType

_CACHE = {}


def build_nc():
    if "nc" in _CACHE:
        return _CACHE["nc"]
    nc = bacc.Bacc("TRN2", target_bir_lowering=False)

    xT = nc.dram_tensor("xT", (P, DC, S), BF16, kind="ExternalInput")
    wq = nc.dram_tensor("wq", (HL, P, DC, P), BF16, kind="ExternalInput")
    wk = nc.dram_tensor("wk", (P, DC, KVJ), BF16, kind="ExternalInput")
    wv = nc.dram_tensor("wv", (P, DC, KVJ), BF16, kind="ExternalInput")
    wo = nc.dram_tensor("wo", (HL, P, HID), BF16, kind="ExternalInput")
    cos_q = nc.dram_tensor("cos_q", (P, S), BF16, kind="ExternalInput")
    sin_q = nc.dram_tensor("sin_q", (P, S), BF16, kind="ExternalInput")
    cos_k = nc.dram_tensor("cos_k", (P, S), BF16, kind="ExternalInput")
    sin_k = nc.dram_tensor("sin_k", (P, S), BF16, kind="ExternalInput")
    pmat = nc.dram_tensor("pmat", (P, P), BF16, kind="ExternalInput")
    ones = nc.dram_tensor("ones", (P, 1), BF16, kind="ExternalInput")
    out = nc.dram_tensor("out", (S, HID), F32, kind="ExternalOutput")

    with tile.TileContext(nc) as tc:
        with (
            tc.tile_pool(name="consts", bufs=1) as consts,
            tc.tile_pool(name="kt", bufs=1) as ktp,
            tc.tile_pool(name="vt", bufs=1) as vtp,
            tc.tile_pool(name="qh", bufs=1) as qhp,
            tc.tile_pool(name="wop", bufs=1) as wop,
        ):
            pm_t = consts.tile([P, P], BF16)
            nc.sync.dma_start(pm_t[:], pmat.ap())
            ones_t = consts.tile([P, 1], BF16)
            nc.sync.dma_start(ones_t[:], ones.ap())
            kt = ktp.tile([P, KVL, S], BF16)
            vt = vtp.tile([P, NKC, KVJ], BF16)
            qh = [qhp.tile([P, S], BF16, name=f"qh{h}") for h in range(HL)]
            wo_sb = [wop.tile([P, HID], BF16, name=f"wo{h}") for h in range(HL)]

            def rope_stage1(raw_ps, work, w, tagp, u):
                # ACT: psum -> sbuf bf16 copy of the raw projection
                raw_sb = work.tile([P, w], BF16, tag=f"{tagp}raw", name=f"rraw{u}")
                nc.scalar.copy(raw_sb[:], raw_ps)
                return raw_sb

            def rope_stage2(raw_sb, swp_ps, cos_sl, sin_sl, dst, work, w, tagp, u):
                # PE: swap matmul; ACT: copy out; DVE: cos/sin multiply-add
                nc.tensor.matmul(swp_ps, lhsT=pm_t[:], rhs=raw_sb[:],
                                 start=True, stop=True)
                swp_sb = work.tile([P, w], BF16, tag=f"{tagp}swp", name=f"rswp{u}")
                nc.scalar.copy(swp_sb[:], swp_ps)
                ta = work.tile([P, w], BF16, tag=f"{tagp}a", name=f"ra{u}")
                nc.vector.tensor_tensor(ta[:], raw_sb[:], cos_sl, AL.mult)
                tb = work.tile([P, w], BF16, tag=f"{tagp}b", name=f"rb{u}")
                nc.vector.tensor_tensor(tb[:], swp_sb[:], sin_sl, AL.mult)
                nc.vector.tensor_tensor(dst, ta[:], tb[:], AL.add)

            # ---- Phase KV: K^T (roped) and V, streaming xT tiles ----
            with (
                tc.tile_pool(name="xin", bufs=2) as xin,
                tc.tile_pool(name="ktab", bufs=1) as ktab,
                tc.tile_pool(name="wkp", bufs=1) as wkp,
                tc.tile_pool(name="ropeK", bufs=2) as ropeK,
                tc.tile_pool(name="ppP", bufs=2, space="PSUM") as ppP,
                tc.tile_pool(name="ppS", bufs=2, space="PSUM") as ppS,
                tc.tile_pool(name="ppV", bufs=2, space="PSUM") as ppV,
            ):
                ck_t = ktab.tile([P, S], BF16, name="cosk")
                nc.sync.dma_start(ck_t[:], cos_k.ap())
                sk_t = ktab.tile([P, S], BF16, name="sink")
                nc.sync.dma_start(sk_t[:], sin_k.ap())
                wk_sb = wkp.tile([P, DC, KVJ], BF16, name="wk")
                wv_sb = wkp.tile([P, DC, KVJ], BF16, name="wv")
                xt0 = xin.tile([P, DC, ST], BF16, tag="xt", name="xt0")
                nc.sync.dma_start(xt0[:], xT.ap()[:, :, 0:ST])
                nc.sync.dma_start(wk_sb[:], wk.ap())
                nc.sync.dma_start(wv_sb[:], wv.ap())
                pend = []  # deferred rope stage2: (raw_sb, st, jc)
                for st in range(NST):
                    if st == 0:
                        xt = xt0
                    else:
                        xt = xin.tile([P, DC, ST], BF16, tag="xt", name=f"xt{st}")
                        nc.sync.dma_start(xt[:], xT.ap()[:, :, st * ST:(st + 1) * ST])
                    for jc in range(KVL):
                        pk = ppP.tile([P, ST], F32, tag="pk")
                        for dc in range(DC):
                            nc.tensor.matmul(
                                pk[:],
                                lhsT=wk_sb[:, dc, jc * P:(jc + 1) * P],
                                rhs=xt[:, dc, :],
                                start=(dc == 0),
                                stop=(dc == DC - 1),
                            )
                        raw_sb = rope_stage1(pk[:], ropeK, ST, "k", f"{st}_{jc}")
                        pend.append((raw_sb, st, jc))
                        if len(pend) > 1:
                            r, pst, pjc = pend.pop(0)
                            sl = slice(pst * ST, (pst + 1) * ST)
                            swp = ppS.tile([P, ST], F32, tag="swp")
                            rope_stage2(r, swp[:], ck_t[:, sl], sk_t[:, sl],
                                        kt[:, pjc, sl], ropeK, ST, "k", f"{pst}_{pjc}")
                    pv = ppV.tile([P, NST, KVJ], F32, tag="pv")
                    for si in range(NST):
                        for dc in range(DC):
                            nc.tensor.matmul(
                                pv[:, si, :],
                                lhsT=xt[:, dc, si * P:(si + 1) * P],
                                rhs=wv_sb[:, dc, :],
                                start=(dc == 0),
                                stop=(dc == DC - 1),
                            )
                    nc.scalar.copy(vt[:, st * NST:(st + 1) * NST, :], pv[:])
                while pend:
                    r, pst, pjc = pend.pop(0)
                    sl = slice(pst * ST, (pst + 1) * ST)
                    swp = ppS.tile([P, ST], F32, tag="swp")
                    rope_stage2(r, swp[:], ck_t[:, sl], sk_t[:, sl],
                                kt[:, pjc, sl], ropeK, ST, "k", f"{pst}_{pjc}")

            # ---- Phase Q: all 8 heads, roped ----
            with (
                tc.tile_pool(name="xinq", bufs=2) as xinq,
                tc.tile_pool(name="qtab", bufs=1) as qtab,
                tc.tile_pool(name="wqp", bufs=1) as wqp,
                tc.tile_pool(name="ropeQ", bufs=2) as ropeQ,
                tc.tile_pool(name="ppQ", bufs=2, space="PSUM") as ppQ,
                tc.tile_pool(name="ppSQ", bufs=2, space="PSUM") as ppSQ,
            ):
                cq_t = qtab.tile([P, S], BF16, name="cosq")
                nc.sync.dma_start(cq_t[:], cos_q.ap())
                sq_t = qtab.tile([P, S], BF16, name="sinq")
                nc.sync.dma_start(sq_t[:], sin_q.ap())
                wq_sb = [wqp.tile([P, DC, P], BF16, name=f"wq{h}") for h in range(HL)]
                xtq0 = xinq.tile([P, DC, ST], BF16, tag="xtq", name="xtq0")
                nc.sync.dma_start(xtq0[:], xT.ap()[:, :, 0:ST])
                for h in range(HL):
                    nc.sync.dma_start(wq_sb[h][:], wq.ap()[h])
                # prefetch wo for phase B while xT streams
                for h in range(HL):
                    nc.sync.dma_start(wo_sb[h][:], wo.ap()[h])
                pend = []
                for st in range(NST):
                    if st == 0:
                        xt = xtq0
                    else:
                        xt = xinq.tile([P, DC, ST], BF16, tag="xtq", name=f"xtq{st}")
                        nc.sync.dma_start(xt[:], xT.ap()[:, :, st * ST:(st + 1) * ST])
                    for h in range(HL):
                        pq = ppQ.tile([P, ST], F32, tag="pq")
                        for dc in range(DC):
                            nc.tensor.matmul(
                                pq[:],
                                lhsT=wq_sb[h][:, dc, :],
                                rhs=xt[:, dc, :],
                                start=(dc == 0),
                                stop=(dc == DC - 1),
                            )
                        raw_sb = rope_stage1(pq[:], ropeQ, ST, "q", f"{st}_{h}")
                        pend.append((raw_sb, st, h))
                        if len(pend) > 1:
                            r, pst, ph = pend.pop(0)
                            sl = slice(pst * ST, (pst + 1) * ST)
                            swp = ppSQ.tile([P, ST], F32, tag="swq")
                            rope_stage2(r, swp[:], cq_t[:, sl], sq_t[:, sl],
                                        qh[ph][:, sl], ropeQ, ST, "q", f"{pst}_{ph}")
                while pend:
                    r, pst, ph = pend.pop(0)
                    sl = slice(pst * ST, (pst + 1) * ST)
                    swp = ppSQ.tile([P, ST], F32, tag="swq")
                    rope_stage2(r, swp[:], cq_t[:, sl], sq_t[:, sl],
                                qh[ph][:, sl], ropeQ, ST, "q", f"{pst}_{ph}")

            # ---- Phase B: attention + fused o_proj per q-tile ----
            with (
                tc.tile_pool(name="ptp", bufs=2) as ptp,
                tc.tile_pool(name="tree", bufs=1) as treep,
                tc.tile_pool(name="attp", bufs=2) as attp,
                tc.tile_pool(name="nrm", bufs=2) as nrmp,
                tc.tile_pool(name="outp", bufs=3) as outp,
                tc.tile_pool(name="ppSc", bufs=2, space="PSUM") as ppSc,
                tc.tile_pool(name="ppAv", bufs=2, space="PSUM") as ppAv,
                tc.tile_pool(name="ppO", bufs=2, space="PSUM") as ppO,
            ):
                def emit_den(u):
                    # den borrows a scores-ring psum slot ([1, ST] of it)
                    den_t = ppSc.tile([P, 2 * ST], F32, tag="scores", name=f"den{u['id']}")
                    den = den_t[0:1, 0:ST]
                    nc.tensor.matmul(den, lhsT=ones_t[:], rhs=u["t4"][:],
                                     start=True, stop=True)
                    u["den"] = den

                def emit_normalize(u):
                    r_row = nrmp.tile([1, ST], F32, tag="rrow", name=f"rr{u['id']}")
                    nc.vector.reciprocal(r_row[:], u["den"])
                    rb = nrmp.tile([P, ST], F32, tag="rb", name=f"rb{u['id']}")
                    nc.gpsimd.partition_broadcast(rb[:], r_row[:])
                    nc.vector.tensor_tensor(u["att"][:], u["av"][:], rb[:], AL.mult)

                def emit_oproj(qt, att_set):
                    for qc in range(ST // P):
                        for ot in range(HID // ST):
                            po = ppO.tile([P, ST], F32, tag="po")
                            for h in range(HL):
                                nc.tensor.matmul(
                                    po[:],
                                    lhsT=att_set[h][:, qc * P:(qc + 1) * P],
                                    rhs=wo_sb[h][:, ot * ST:(ot + 1) * ST],
                                    start=(h == 0),
                                    stop=(h == HL - 1),
                                )
                            out_t = outp.tile([P, ST], F32, tag="outt")
                            nc.vector.tensor_copy(out_t[:], po[:])
                            nc.sync.dma_start(
                                out.ap()[qt * ST + qc * P:qt * ST + (qc + 1) * P,
                                         ot * ST:(ot + 1) * ST],
                                out_t[:],
                            )

                prev = None       # unit whose den/normalize is pending
                prev_oproj = None  # (qt, att_set) pending o_proj
                for qt in range(NST):
                    qsl = slice(qt * ST, (qt + 1) * ST)
                    att_set = [attp.tile([P, ST], BF16, tag=f"att{h}",
                                         name=f"att{qt}_{h}") for h in range(HL)]
                    for h in range(HL):
                        uid = qt * HL + h
                        kv = h // (HL // KVL)
                        av = ppAv.tile([P, ST], F32, tag="av")
                        u = {"id": uid, "av": av, "att": att_set[h]}
                        pt = [None] * 8
                        sc = [None] * 8
                        avq = []  # (kc, pt_tile, i)

                        def flush_av(n):
                            while len(avq) > n:
                                kc, ptt, i = avq.pop(0)
                                nc.tensor.matmul(
                                    av[:],
                                    lhsT=vt[:, kc, kv * P:(kv + 1) * P],
                                    rhs=ptt[:, i * ST:(i + 1) * ST],
                                    start=(kc == 0),
                                    stop=(kc == NKC - 1),
                                )

                        for kp in range(8):
                            sc_ps = ppSc.tile([P, 2 * ST], F32, tag="scores")
                            for i in range(2):
                                kc = kp * 2 + i
                                nc.tensor.matmul(
                                    sc_ps[:, i * ST:(i + 1) * ST],
                                    lhsT=kt[:, kv, kc * P:(kc + 1) * P],
                                    rhs=qh[h][:, qsl],
                                    start=True,
                                    stop=True,
                                )
                            if kp == 1:
                                # previous unit's denominator + o_proj of the
                                # previous q-tile go here, between score mms
                                if prev is not None:
                                    emit_den(prev)
                                    emit_normalize(prev)
                                if prev_oproj is not None and h == 0:
                                    emit_oproj(*prev_oproj)
                                    prev_oproj = None
                            pt[kp] = ptp.tile([P, 2 * ST], BF16, tag=f"pt{kp}",
                                              name=f"pt{uid}_{kp}")
                            nc.scalar.activation(pt[kp][:], sc_ps[:], AF.Exp)
                            avq.append((2 * kp, pt[kp], 0))
                            avq.append((2 * kp + 1, pt[kp], 1))
                            flush_av(4)  # av trails scores by 2 kp
                        flush_av(0)
                        # denominator sum tree on DVE (bf16 2x)
                        t1 = [treep.tile([P, 2 * ST], BF16, tag=f"t1{j}",
                                         name=f"t1_{uid}_{j}") for j in range(4)]
                        for j in range(4):
                            nc.vector.tensor_tensor(
                                t1[j][:], pt[2 * j][:], pt[2 * j + 1][:], AL.add
                            )
                        t2 = [treep.tile([P, 2 * ST], BF16, tag=f"t2{j}",
                                         name=f"t2_{uid}_{j}") for j in range(2)]
                        for j in range(2):
                            nc.vector.tensor_tensor(
                                t2[j][:], t1[2 * j][:], t1[2 * j + 1][:], AL.add
                            )
                        t3 = treep.tile([P, 2 * ST], BF16, tag="t3", name=f"t3_{uid}")
                        nc.vector.tensor_tensor(t3[:], t2[0][:], t2[1][:], AL.add)
                        t4 = treep.tile([P, ST], BF16, tag="t4", name=f"t4_{uid}")
                        nc.vector.tensor_tensor(
                            t4[:], t3[:, 0:ST], t3[:, ST:2 * ST], AL.add
                        )
                        u["t4"] = t4
                        prev = u
                    prev_oproj = (qt, att_set)
                # drain: last unit's den/normalize, last q-tile's o_proj
                emit_den(prev)
                emit_normalize(prev)
                emit_oproj(*prev_oproj)

    nc.compile()
    _CACHE["nc"] = nc
    return nc


def _host_inputs(x, Wq, Wk, Wv, Wo):
    """Build the 8 per-core input maps (numpy only)."""
    bf = ml_dtypes.bfloat16

    # rope tables: row j uses frequency j%64
    inv_ts = ROPE_THETA ** (-2.0 * np.arange(D // 2) / D)
    inv_full = np.concatenate([inv_ts, inv_ts])
    pos = np.arange(S, dtype=np.float64)
    ang = inv_full[:, None] * pos[None, :]
    cos_k = np.cos(ang).astype(bf)
    sin_k = np.sin(ang).astype(bf)
    scale = 1.0 / math.sqrt(D)
    cos_q = (np.cos(ang) * scale).astype(bf)
    sin_q = (np.sin(ang) * scale).astype(bf)

    pmat = np.zeros((P, P), np.float32)  # lhsT: swap[i] = -q[i+64] (i<64), +q[i-64]
    for i in range(64):
        pmat[i + 64, i] = -1.0
        pmat[i, i + 64] = 1.0
    pmat = pmat.astype(bf)
    ones = np.ones((P, 1), bf)

    in_maps = []
    for c in range(8):
        b, hg = c // 2, c % 2
        hsl = slice(hg * HL, (hg + 1) * HL)
        kvsl = slice(hg * KVL, (hg + 1) * KVL)
        xTb = np.ascontiguousarray(
            x[b].T.reshape(DC, P, S).transpose(1, 0, 2)
        ).astype(bf)  # [p, dc, s]
        wq_sw = np.ascontiguousarray(
            Wq[:, hsl, :].reshape(DC, P, HL, P).transpose(2, 1, 0, 3)
        ).astype(bf)  # [h, p, dc, j]
        wk_sw = np.ascontiguousarray(
            Wk[:, kvsl, :].reshape(DC, P, KVJ).transpose(1, 0, 2)
        ).astype(bf)  # [p, dc, j]
        wv_sw = np.ascontiguousarray(
            Wv[:, kvsl, :].reshape(DC, P, KVJ).transpose(1, 0, 2)
        ).astype(bf)
        wo_sw = np.ascontiguousarray(Wo[hsl]).astype(bf)  # [h, j(=d), o]
        in_maps.append(
            {
                "xT": xTb,
                "wq": wq_sw,
                "wk": wk_sw,
                "wv": wv_sw,
                "wo": wo_sw,
                "cos_q": cos_q,
                "sin_q": sin_q,
                "cos_k": cos_k,
                "sin_k": sin_k,
                "pmat": pmat,
                "ones": ones,
            }
        )
    return in_maps


def kernel(x, Wq, Wk, Wv, Wo, _trace=False):
    x, Wq, Wk, Wv, Wo = (np.asarray(a, dtype=np.float32) for a in (x, Wq, Wk, Wv, Wo))
    nc = build_nc()
    in_maps = _host_inputs(x, Wq, Wk, Wv, Wo)
    res = run_bass_kernel_spmd(nc, in_maps, core_ids=list(range(8)), trace=_trace)
    out = np.empty((B, S, HID), np.float32)
    for b in range(B):
        out[b] = res.results[2 * b]["out"]
        out[b] += res.results[2 * b + 1]["out"]
    if _trace:
        kernel.last_results = res
    return out


# revision 27
# speedup vs baseline: 1.3505x; 1.1880x over previous
"""Fused multi-head attention (RoPE + GQA + softmax + o_proj) on 8 Trainium2 cores.

Sharding: core c handles batch b = c//2 and head-group hg = c%2
(8 q-heads / 2 kv-heads), ALL 2048 queries.  Each core computes K/V for
only its kv heads, attention for its 8 q heads, and a PARTIAL o_proj
(contracted over its heads).  The host sums the two partial outputs per
batch (the "all-reduce after o_proj" of the tensor-parallel sharding).
Per-core matmul work is exactly 1/8 of the model total.

Everything runs in bf16 (1 cycle/row on the PE, same as f32r, but 2x on
DVE and half the DMA/SBUF), accumulating in f32 PSUM.

Pipelining (PE program order is execution order per engine):
 - rope's swap matmul for iteration u is emitted inside iteration u+1 so
   the PE never waits on the ACT psum->sbuf copy.
 - attention unit u = (qt, h): av matmuls trail the score matmuls by 2
   kp-steps so the ACT exp pipeline stays ahead of the PE.
 - the denominator matmul + normalize of unit u are emitted inside unit
   u+1 (tree latency hidden); den borrows a scores-ring PSUM slot.
 - o_proj of q-tile qt is emitted inside unit (qt+1, h0) so ACT/DVE of
   the next tile's units overlap its matmuls.

Per-core layouts (partition dim first):
  xT  [128, 16, S]   x[b]^T swizzled: partition=d%128, (dchunk, s)  bf16
  kt  [128, 2, S]    roped K, partition=d%128 of the kv head        bf16
  vt  [128, 16, 256] V, partition=s%128, (schunk, j of 2 kv heads)  bf16
  qh  [8][128, S]    roped Q per head, partition=d%128              bf16
  att [2][8][128, 512]  per q-tile: attention out, partition=j      bf16
"""

import os
import sys

sys.path.insert(0, "/opt/trn_rl_repo")

import math

import numpy as np
import ml_dtypes

import concourse.bass as bass
import concourse.mybir as mybir
import concourse.tile as tile
from concourse import bacc
from concourse.bass_utils import run_bass_kernel_spmd

P = 128
B, S, HID = 4, 2048, 2048
H, HKV, D = 16, 4, 128
HL = H // 2  # 8 q heads per core
KVL = HKV // 2  # 2 kv heads per core
DC = HID // P  # 16
KVJ = KVL * D  # 256
ST = 512  # s-tile for projections; also q-tile for attention
NST = S // ST  # 4
NKC = S // P  # 16 key chunks
ROPE_THETA = 10000.0

F32 = mybir.dt.float32
BF16 = mybir.dt.bfloat16
AL = mybir.AluOpType
AF = mybir.ActivationFunctionType

_CACHE = {}


def build_nc():
    if "nc" in _CACHE:
        return _CACHE["nc"]
    phases = os.environ.get("KERNEL_PHASES", "all")  # kv | kvq | all
    nc = bacc.Bacc("TRN2", target_bir_lowering=False)

    xT = nc.dram_tensor("xT", (P, DC, S), BF16, kind="ExternalInput")
    wq = nc.dram_tensor("wq", (HL, P, DC, P), BF16, kind="ExternalInput")
    wk = nc.dram_tensor("wk", (P, DC, KVJ), BF16, kind="ExternalInput")
    wv = nc.dram_tensor("wv", (P, DC, KVJ), BF16, kind="ExternalInput")
    wo = nc.dram_tensor("wo", (HL, P, HID), BF16, kind="ExternalInput")
    cos_q = nc.dram_tensor("cos_q", (P, S), BF16, kind="ExternalInput")
    sin_q = nc.dram_tensor("sin_q", (P, S), BF16, kind="ExternalInput")
    cos_k = nc.dram_tensor("cos_k", (P, S), BF16, kind="ExternalInput")
    sin_k = nc.dram_tensor("sin_k", (P, S), BF16, kind="ExternalInput")
    pmat = nc.dram_tensor("pmat", (P, P), BF16, kind="ExternalInput")
    ones = nc.dram_tensor("ones", (P, 1), BF16, kind="ExternalInput")
    out = nc.dram_tensor("out", (S, HID), F32, kind="ExternalOutput")

    with tile.TileContext(nc) as tc:
        with (
            tc.tile_pool(name="consts", bufs=1) as consts,
            tc.tile_pool(name="kt", bufs=1) as ktp,
            tc.tile_pool(name="vt", bufs=1) as vtp,
            tc.tile_pool(name="qh", bufs=1) as qhp,
            tc.tile_pool(name="wop", bufs=1) as wop,
        ):
            pm_t = consts.tile([P, P], BF16)
            nc.sync.dma_start(pm_t[:], pmat.ap())
            ones_t = consts.tile([P, 1], BF16)
            nc.sync.dma_start(ones_t[:], ones.ap())
            kt = ktp.tile([P, KVL, S], BF16)
            vt = vtp.tile([P, NKC, KVJ], BF16)
            qh = [qhp.tile([P, S], BF16, name=f"qh{h}") for h in range(HL)]
            wo_sb = [wop.tile([P, HID], BF16, name=f"wo{h}") for h in range(HL)]

            def rope_stage1(raw_ps, work, w, tagp, u):
                # ACT: psum -> sbuf bf16 copy of the raw projection
                raw_sb = work.tile([P, w], BF16, tag=f"{tagp}raw", name=f"rraw{u}")
                nc.scalar.copy(raw_sb[:], raw_ps)
                return raw_sb

            def rope_stage2(raw_sb, swp_ps, cos_sl, sin_sl, dst, work, w, tagp, u):
                # PE: swap matmul; ACT: copy out; DVE: cos/sin multiply-add
                nc.tensor.matmul(swp_ps, lhsT=pm_t[:], rhs=raw_sb[:],
                                 start=True, stop=True)
                swp_sb = work.tile([P, w], BF16, tag=f"{tagp}swp", name=f"rswp{u}")
                nc.scalar.copy(swp_sb[:], swp_ps)
                ta = work.tile([P, w], BF16, tag=f"{tagp}a", name=f"ra{u}")
                nc.vector.tensor_tensor(ta[:], raw_sb[:], cos_sl, AL.mult)
                tb = work.tile([P, w], BF16, tag=f"{tagp}b", name=f"rb{u}")
                nc.vector.tensor_tensor(tb[:], swp_sb[:], sin_sl, AL.mult)
                nc.vector.tensor_tensor(dst, ta[:], tb[:], AL.add)

            # ---- Phase P: K/V projections then Q, one scope so the Q-phase
            # DMAs (x re-stream, wq, rope tables) are issued while KV computes.
            with (
                tc.tile_pool(name="xin", bufs=2) as xin,
                tc.tile_pool(name="tabs", bufs=1) as tabs,
                tc.tile_pool(name="wkp", bufs=1) as wkp,
                tc.tile_pool(name="wqp", bufs=1) as wqp,
                tc.tile_pool(name="ropeP", bufs=2) as ropeP,
                tc.tile_pool(name="ppP", bufs=2, space="PSUM") as ppP,
                tc.tile_pool(name="ppS", bufs=2, space="PSUM") as ppS,
                tc.tile_pool(name="ppV", bufs=2, space="PSUM") as ppV,
            ):
                # startup order: wk + first x halves first so the PE starts ASAP
                wk_sb = wkp.tile([P, DC, KVJ], BF16, name="wk")
                nc.sync.dma_start(wk_sb[:], wk.ap())
                xt0 = [xin.tile([P, DC // 2, ST], BF16, tag=f"xh{i}", name=f"xt0_{i}")
                       for i in range(2)]
                for i in range(2):
                    nc.sync.dma_start(
                        xt0[i][:], xT.ap()[:, i * (DC // 2):(i + 1) * (DC // 2), 0:ST]
                    )
                wv_sb = wkp.tile([P, DC, KVJ], BF16, name="wv")
                nc.sync.dma_start(wv_sb[:], wv.ap())
                ck_t = tabs.tile([P, S], BF16, name="cosk")
                nc.sync.dma_start(ck_t[:], cos_k.ap())
                sk_t = tabs.tile([P, S], BF16, name="sink")
                nc.sync.dma_start(sk_t[:], sin_k.ap())

                def xslice(xt, dc, s0=0, s1=ST):
                    return xt[dc // (DC // 2)][:, dc % (DC // 2), s0:s1]

                pend = []  # deferred rope stage2

                def flush_pend(n):
                    while len(pend) > n:
                        r, cs, ss, dst, tagp, uid = pend.pop(0)
                        pp = ppS.tile([P, ST], F32, tag="swp")
                        rope_stage2(r, pp[:], cs, ss, dst, ropeP, ST, tagp, uid)

                for st in range(NST):
                    if st == 0:
                        xt = xt0
                    else:
                        xt = [xin.tile([P, DC // 2, ST], BF16, tag=f"xh{i}",
                                       name=f"xt{st}_{i}") for i in range(2)]
                        for i in range(2):
                            nc.sync.dma_start(
                                xt[i][:],
                                xT.ap()[:, i * (DC // 2):(i + 1) * (DC // 2),
                                        st * ST:(st + 1) * ST],
                            )
                    sl = slice(st * ST, (st + 1) * ST)
                    for jc in range(KVL):
                        pk = ppP.tile([P, ST], F32, tag="pk")
                        for dc in range(DC):
                            nc.tensor.matmul(
                                pk[:],
                                lhsT=wk_sb[:, dc, jc * P:(jc + 1) * P],
                                rhs=xslice(xt, dc),
                                start=(dc == 0),
                                stop=(dc == DC - 1),
                            )
                        raw_sb = rope_stage1(pk[:], ropeP, ST, "r", f"k{st}_{jc}")
                        pend.append((raw_sb, ck_t[:, sl], sk_t[:, sl],
                                     kt[:, jc, sl], "r", f"k{st}_{jc}"))
                        flush_pend(1)
                    pv = ppV.tile([P, NST, KVJ], F32, tag="pv")
                    for si in range(NST):
                        for dc in range(DC):
                            nc.tensor.matmul(
                                pv[:, si, :],
                                lhsT=xslice(xt, dc, si * P, (si + 1) * P),
                                rhs=wv_sb[:, dc, :],
                                start=(dc == 0),
                                stop=(dc == DC - 1),
                            )
                    nc.scalar.copy(vt[:, st * NST:(st + 1) * NST, :], pv[:])
                    if st == NST - 1:
                        # issue Q-phase DMAs now: re-stream x, weights, tables.
                        # Only 2 xtq tiles up front (ring depth) so later DMAs
                        # aren't FIFO-blocked behind a WAR-held transfer.
                        xtq = [[xin.tile([P, DC // 2, ST], BF16, tag=f"xh{i}",
                                         name=f"xtq{s}_{i}") for i in range(2)]
                               for s in range(NST)]
                        wq_sb = [wqp.tile([P, DC, P], BF16, name=f"wq{h}")
                                 for h in range(HL)]
                        cq_t = tabs.tile([P, S], BF16, name="cosq")
                        sq_t = tabs.tile([P, S], BF16, name="sinq")

                        def dma_xtq(s):
                            for i in range(2):
                                nc.sync.dma_start(
                                    xtq[s][i][:],
                                    xT.ap()[:, i * (DC // 2):(i + 1) * (DC // 2),
                                            s * ST:(s + 1) * ST],
                                )

                        dma_xtq(0)
                        for h in range(2):
                            nc.sync.dma_start(wq_sb[h][:], wq.ap()[h])
                        dma_xtq(1)
                        nc.sync.dma_start(cq_t[:], cos_q.ap())
                        nc.sync.dma_start(sq_t[:], sin_q.ap())
                        for h in range(2, HL):
                            nc.sync.dma_start(wq_sb[h][:], wq.ap()[h])

                # ---- Q projections, all 8 heads ----
                for st in range(NST if phases != "kv" else 0):
                    xt = xtq[st]
                    if st + 2 < NST:
                        dma_xtq(st + 2)
                    sl = slice(st * ST, (st + 1) * ST)
                    for h in range(HL):
                        pq = ppP.tile([P, ST], F32, tag="pk")
                        for dc in range(DC):
                            nc.tensor.matmul(
                                pq[:],
                                lhsT=wq_sb[h][:, dc, :],
                                rhs=xslice(xt, dc),
                                start=(dc == 0),
                                stop=(dc == DC - 1),
                            )
                        raw_sb = rope_stage1(pq[:], ropeP, ST, "r", f"q{st}_{h}")
                        pend.append((raw_sb, cq_t[:, sl], sq_t[:, sl],
                                     qh[h][:, sl], "r", f"q{st}_{h}"))
                        flush_pend(1)
                flush_pend(0)

            # ---- Phase B: attention + fused o_proj per q-tile ----
            with (
                tc.tile_pool(name="ptp", bufs=2) as ptp,
                tc.tile_pool(name="tree", bufs=1) as treep,
                tc.tile_pool(name="attp", bufs=2) as attp,
                tc.tile_pool(name="nrm", bufs=2) as nrmp,
                tc.tile_pool(name="outp", bufs=3) as outp,
                tc.tile_pool(name="ppSc", bufs=1, space="PSUM") as ppSc,
                tc.tile_pool(name="ppAv", bufs=2, space="PSUM") as ppAv,
                tc.tile_pool(name="ppO", bufs=2, space="PSUM") as ppO,
            ):
                GW = 4 * ST  # score/exp group: 4 key-chunks
                NG = S // GW  # 4 groups per unit

                def emit_den(u):
                    # den borrows a po-ring psum slot ([1, ST] of it)
                    den_t = ppO.tile([P, ST], F32, tag="po", name=f"den{u['id']}")
                    den = den_t[0:1, 0:ST]
                    nc.tensor.matmul(den, lhsT=ones_t[:], rhs=u["t4"][:],
                                     start=True, stop=True)
                    u["den"] = den

                def emit_normalize(u):
                    r_row = nrmp.tile([1, ST], F32, tag="rrow", name=f"rr{u['id']}")
                    nc.vector.reciprocal(r_row[:], u["den"])
                    rb = nrmp.tile([P, ST], F32, tag="rb", name=f"rb{u['id']}")
                    nc.gpsimd.partition_broadcast(rb[:], r_row[:])
                    nc.vector.tensor_tensor(u["att"][:], u["av"][:], rb[:], AL.mult)

                oq = []  # pending o_proj chains, emitted one per score group

                def emit_ochain(qt, att_set, qc, ot):
                    po = ppO.tile([P, ST], F32, tag="po")
                    for h in range(HL):
                        nc.tensor.matmul(
                            po[:],
                            lhsT=att_set[h][:, qc * P:(qc + 1) * P],
                            rhs=wo_sb[h][:, ot * ST:(ot + 1) * ST],
                            start=(h == 0),
                            stop=(h == HL - 1),
                        )
                    out_t = outp.tile([P, ST], F32, tag="outt")
                    nc.vector.tensor_copy(out_t[:], po[:])
                    nc.sync.dma_start(
                        out.ap()[qt * ST + qc * P:qt * ST + (qc + 1) * P,
                                 ot * ST:(ot + 1) * ST],
                        out_t[:],
                    )

                def pop_ochain(n=1):
                    for _ in range(min(n, len(oq))):
                        emit_ochain(*oq.pop(0))

                for h in range(HL):
                    nc.sync.dma_start(wo_sb[h][:], wo.ap()[h])
                prev = None  # unit whose den/normalize is pending
                avq = []  # pending AV matmuls: (av_tile, kc, pt_tile, i, kv)

                def flush_av(n):
                    while len(avq) > n:
                        avt, kc, ptt, i, fkv = avq.pop(0)
                        nc.tensor.matmul(
                            avt[:],
                            lhsT=vt[:, kc, fkv * P:(fkv + 1) * P],
                            rhs=ptt[:, i * ST:(i + 1) * ST],
                            start=(kc == 0),
                            stop=(kc == NKC - 1),
                        )

                for qt in range(NST if phases == "all" else 0):
                    qsl = slice(qt * ST, (qt + 1) * ST)
                    att_set = [attp.tile([P, ST], BF16, tag=f"att{h}",
                                         name=f"att{qt}_{h}") for h in range(HL)]
                    for h in range(HL):
                        uid = qt * HL + h
                        kv = h // (HL // KVL)
                        av = ppAv.tile([P, ST], F32, tag="av")
                        u = {"id": uid, "av": av, "att": att_set[h]}
                        pt = [None] * 8

                        run = None  # running sum of pt tiles (DVE, bf16 2x)
                        for kp in range(8):
                            sc_ps = ppSc.tile([P, 2 * ST], F32, tag="scores",
                                              bufs=2)
                            for i in range(2):
                                kc = kp * 2 + i
                                nc.tensor.matmul(
                                    sc_ps[:, i * ST:(i + 1) * ST],
                                    lhsT=kt[:, kv, kc * P:(kc + 1) * P],
                                    rhs=qh[h][:, qsl],
                                    start=True,
                                    stop=True,
                                )
                            if kp == 2 and prev is not None:
                                emit_den(prev)
                                emit_normalize(prev)
                                prev = None
                            pt[kp] = ptp.tile([P, 2 * ST], BF16, tag=f"pt{kp}",
                                              name=f"pt{uid}_{kp}")
                            nc.scalar.activation(pt[kp][:], sc_ps[:], AF.Exp)
                            for i in range(2):
                                avq.append((av, kp * 2 + i, pt[kp], i, kv))
                            flush_av(4)  # av trails scores by two kp
                            if kp > 0:
                                nxt = treep.tile([P, 2 * ST], BF16,
                                                 tag=f"rs{kp % 2}",
                                                 name=f"rs_{uid}_{kp}")
                                nc.vector.tensor_tensor(
                                    nxt[:], run[:] if kp > 1 else pt[0][:],
                                    pt[kp][:], AL.add
                                )
                                run = nxt
                            # o_proj chain of the previous q-tile every 4 kp
                            if kp % 4 == 3:
                                pop_ochain(1)
                        t4 = treep.tile([P, ST], BF16, tag="t4", name=f"t4_{uid}")
                        nc.vector.tensor_tensor(
                            t4[:], run[:, 0:ST], run[:, ST:2 * ST], AL.add
                        )
                        u["t4"] = t4
                        prev = u
                    for qc in range(ST // P):
                        for ot in range(HID // ST):
                            oq.append((qt, att_set, qc, ot))
                # drain: pending AVs, last unit's den/normalize, last o_proj
                flush_av(0)
                if prev is not None:
                    emit_den(prev)
                    emit_normalize(prev)
                pop_ochain(len(oq))

    nc.compile()
    _CACHE["nc"] = nc
    return nc


def _host_inputs(x, Wq, Wk, Wv, Wo):
    """Build the 8 per-core input maps (numpy only)."""
    bf = ml_dtypes.bfloat16

    # rope tables: row j uses frequency j%64
    inv_ts = ROPE_THETA ** (-2.0 * np.arange(D // 2) / D)
    inv_full = np.concatenate([inv_ts, inv_ts])
    pos = np.arange(S, dtype=np.float64)
    ang = inv_full[:, None] * pos[None, :]
    cos_k = np.cos(ang).astype(bf)
    sin_k = np.sin(ang).astype(bf)
    scale = 1.0 / math.sqrt(D)
    cos_q = (np.cos(ang) * scale).astype(bf)
    sin_q = (np.sin(ang) * scale).astype(bf)

    pmat = np.zeros((P, P), np.float32)  # lhsT: swap[i] = -q[i+64] (i<64), +q[i-64]
    for i in range(64):
        pmat[i + 64, i] = -1.0
        pmat[i, i + 64] = 1.0
    pmat = pmat.astype(bf)
    ones = np.ones((P, 1), bf)

    in_maps = []
    for c in range(8):
        b, hg = c // 2, c % 2
        hsl = slice(hg * HL, (hg + 1) * HL)
        kvsl = slice(hg * KVL, (hg + 1) * KVL)
        xTb = np.ascontiguousarray(
            x[b].T.reshape(DC, P, S).transpose(1, 0, 2)
        ).astype(bf)  # [p, dc, s]
        wq_sw = np.ascontiguousarray(
            Wq[:, hsl, :].reshape(DC, P, HL, P).transpose(2, 1, 0, 3)
        ).astype(bf)  # [h, p, dc, j]
        wk_sw = np.ascontiguousarray(
            Wk[:, kvsl, :].reshape(DC, P, KVJ).transpose(1, 0, 2)
        ).astype(bf)  # [p, dc, j]
        wv_sw = np.ascontiguousarray(
            Wv[:, kvsl, :].reshape(DC, P, KVJ).transpose(1, 0, 2)
        ).astype(bf)
        wo_sw = np.ascontiguousarray(Wo[hsl]).astype(bf)  # [h, j(=d), o]
        in_maps.append(
            {
                "xT": xTb,
                "wq": wq_sw,
                "wk": wk_sw,
                "wv": wv_sw,
                "wo": wo_sw,
                "cos_q": cos_q,
                "sin_q": sin_q,
                "cos_k": cos_k,
                "sin_k": sin_k,
                "pmat": pmat,
                "ones": ones,
            }
        )
    return in_maps


def kernel(x, Wq, Wk, Wv, Wo, _trace=False):
    x, Wq, Wk, Wv, Wo = (np.asarray(a, dtype=np.float32) for a in (x, Wq, Wk, Wv, Wo))
    nc = build_nc()
    in_maps = _host_inputs(x, Wq, Wk, Wv, Wo)
    res = run_bass_kernel_spmd(nc, in_maps, core_ids=list(range(8)), trace=_trace)
    out = np.empty((B, S, HID), np.float32)
    for b in range(B):
        out[b] = res.results[2 * b]["out"]
        out[b] += res.results[2 * b + 1]["out"]
    if _trace:
        kernel.last_results = res
    return out


# revision 67
# speedup vs baseline: 1.3591x; 1.0064x over previous
"""Fused multi-head attention (RoPE + GQA + softmax + o_proj) on 8 Trainium2 cores.

Sharding: core c handles batch b = c//2 and head-group hg = c%2
(8 q-heads / 2 kv-heads), ALL 2048 queries.  Each core computes K/V for
only its kv heads, attention for its 8 q heads, and a PARTIAL o_proj
(contracted over its heads).  The host sums the two partial outputs per
batch (the "all-reduce after o_proj" of the tensor-parallel sharding).
Per-core matmul work is exactly 1/8 of the model total.

Everything runs in bf16 (1 cycle/row on the PE, same as f32r, but 2x on
DVE and half the DMA/SBUF), accumulating in f32 PSUM.

Pipelining (PE program order is execution order per engine):
 - rope's swap matmul for iteration u is emitted inside iteration u+1 so
   the PE never waits on the ACT psum->sbuf copy.
 - attention unit u = (qt, h): av matmuls trail the score matmuls by 2
   kp-steps so the ACT exp pipeline stays ahead of the PE.
 - the denominator matmul + normalize of unit u are emitted inside unit
   u+1 (tree latency hidden); den borrows a scores-ring PSUM slot.
 - o_proj of q-tile qt is emitted inside unit (qt+1, h0) so ACT/DVE of
   the next tile's units overlap its matmuls.

Per-core layouts (partition dim first):
  xT  [128, 16, S]   x[b]^T swizzled: partition=d%128, (dchunk, s)  bf16
  kt  [128, 2, S]    roped K, partition=d%128 of the kv head        bf16
  vt  [128, 16, 256] V, partition=s%128, (schunk, j of 2 kv heads)  bf16
  qh  [8][128, S]    roped Q per head, partition=d%128              bf16
  att [2][8][128, 512]  per q-tile: attention out, partition=j      bf16
"""

import contextlib
import os
import sys

sys.path.insert(0, "/opt/trn_rl_repo")

import math

import numpy as np
import ml_dtypes

import concourse.bass as bass
import concourse.mybir as mybir
import concourse.tile as tile
from concourse import bacc
from concourse.bass_utils import run_bass_kernel_spmd

P = 128
B, S, HID = 4, 2048, 2048
H, HKV, D = 16, 4, 128
HL = H // 2  # 8 q heads per core
KVL = HKV // 2  # 2 kv heads per core
DC = HID // P  # 16
KVJ = KVL * D  # 256
ST = 512  # s-tile for projections; also q-tile for attention
NST = S // ST  # 4
NKC = S // P  # 16 key chunks
ROPE_THETA = 10000.0

F32 = mybir.dt.float32
BF16 = mybir.dt.bfloat16
AL = mybir.AluOpType
AF = mybir.ActivationFunctionType

_CACHE = {}


def build_nc():
    if "nc" in _CACHE:
        return _CACHE["nc"]
    phases = os.environ.get("KERNEL_PHASES", "all")  # kv | kvq | all
    nc = bacc.Bacc("TRN2", target_bir_lowering=False)

    xT = nc.dram_tensor("xT", (P, DC, S), BF16, kind="ExternalInput")
    wq = nc.dram_tensor("wq", (HL, P, DC, P), BF16, kind="ExternalInput")
    wk = nc.dram_tensor("wk", (P, DC, KVJ), BF16, kind="ExternalInput")
    wv = nc.dram_tensor("wv", (P, DC, KVJ), BF16, kind="ExternalInput")
    wo = nc.dram_tensor("wo", (HL, P, HID), BF16, kind="ExternalInput")
    cos_q = nc.dram_tensor("cos_q", (P, S), BF16, kind="ExternalInput")
    sin_q = nc.dram_tensor("sin_q", (P, S), BF16, kind="ExternalInput")
    cos_k = nc.dram_tensor("cos_k", (P, S), BF16, kind="ExternalInput")
    sin_k = nc.dram_tensor("sin_k", (P, S), BF16, kind="ExternalInput")
    pmat = nc.dram_tensor("pmat", (P, P), BF16, kind="ExternalInput")
    ones = nc.dram_tensor("ones", (P, 1), BF16, kind="ExternalInput")
    out = nc.dram_tensor("out", (S, HID), F32, kind="ExternalOutput")

    with tile.TileContext(nc) as tc:
        with contextlib.ExitStack() as _stk:
            def _pool(name, bufs=1, **kw):
                return _stk.enter_context(tc.tile_pool(name=name, bufs=bufs, **kw))

            consts = _pool("consts")
            ktp = _pool("kt")
            vtp = _pool("vt")
            qhp = _pool("qh")
            wop = _pool("wop")
            wqp = _pool("wqp")
            qtabs = _pool("qtabs")
            xlast = _pool("xlast")
            ropeP = _pool("ropeP", bufs=2)
            pm_t = consts.tile([P, P], BF16)
            nc.sync.dma_start(pm_t[:], pmat.ap())
            ones_t = consts.tile([P, 1], BF16)
            nc.sync.dma_start(ones_t[:], ones.ap())
            kt = ktp.tile([P, KVL, S], BF16)
            vt = vtp.tile([P, NKC, KVJ], BF16)
            qh = [qhp.tile([P, S], BF16, name=f"qh{h}") for h in range(HL)]
            wo_sb = [wop.tile([P, HID], BF16, name=f"wo{h}") for h in range(HL)]

            def rope_stage1(raw_ps, work, w, tagp, u):
                # ACT: psum -> sbuf bf16 copy of the raw projection
                raw_sb = work.tile([P, w], BF16, tag=f"{tagp}raw", name=f"rraw{u}")
                nc.scalar.copy(raw_sb[:], raw_ps)
                return raw_sb

            def rope_stage2(raw_sb, swp_ps, cos_sl, sin_sl, dst, work, w, tagp, u):
                # PE: swap matmul; ACT: copy out; DVE: cos/sin multiply-add
                nc.tensor.matmul(swp_ps, lhsT=pm_t[:], rhs=raw_sb[:],
                                 start=True, stop=True)
                swp_sb = work.tile([P, w], BF16, tag=f"{tagp}swp", name=f"rswp{u}")
                nc.scalar.copy(swp_sb[:], swp_ps)
                ta = work.tile([P, w], BF16, tag=f"{tagp}a", name=f"ra{u}")
                nc.vector.tensor_tensor(ta[:], raw_sb[:], cos_sl, AL.mult)
                tb = work.tile([P, w], BF16, tag=f"{tagp}b", name=f"rb{u}")
                nc.vector.tensor_tensor(tb[:], swp_sb[:], sin_sl, AL.mult)
                nc.vector.tensor_tensor(dst, ta[:], tb[:], AL.add)

            # ---- Phase P: K/V projections then Q, one scope so the Q-phase
            # DMAs (x re-stream, wq, rope tables) are issued while KV computes.
            with contextlib.ExitStack() as _stkP:
                _poolP = lambda name, bufs=1, **kw: _stkP.enter_context(
                    tc.tile_pool(name=name, bufs=bufs, **kw))
                xin = _poolP("xin", bufs=2)
                tabs = _poolP("tabs")
                wkp = _poolP("wkp")
                ppP = _poolP("ppP", bufs=2, space="PSUM")
                ppS = _poolP("ppS", bufs=2, space="PSUM")
                ppV = _poolP("ppV", bufs=2, space="PSUM")
                # startup: quarter-granular first tiles so the PE can start
                # at ~3us and stream behind the DMA arrivals
                QC = DC // 4  # 4 dchunks per quarter
                wk_sb = [wkp.tile([P, DC // 2, KVJ], BF16, name=f"wk{i}")
                         for i in range(2)]
                nc.sync.dma_start(wk_sb[0][:], wk.ap()[:, 0:DC // 2, :])
                xt0 = [xin.tile([P, QC, ST], BF16, tag=f"xq{i}", name=f"xt0_{i}",
                                bufs=1)
                       for i in range(4)]
                nc.sync.dma_start(xt0[0][:], xT.ap()[:, 0:QC, 0:ST])
                nc.sync.dma_start(xt0[1][:], xT.ap()[:, QC:2 * QC, 0:ST])
                nc.sync.dma_start(wk_sb[1][:], wk.ap()[:, DC // 2:DC, :])
                nc.sync.dma_start(xt0[2][:], xT.ap()[:, 2 * QC:3 * QC, 0:ST])
                nc.sync.dma_start(xt0[3][:], xT.ap()[:, 3 * QC:4 * QC, 0:ST])
                wv_sb = wkp.tile([P, DC, KVJ], BF16, name="wv")
                nc.sync.dma_start(wv_sb[:], wv.ap())
                xt1 = [xin.tile([P, DC // 2, ST], BF16, tag=f"xh{i}",
                                name=f"xt1_{i}") for i in range(2)]
                for i in range(2):
                    nc.sync.dma_start(
                        xt1[i][:],
                        xT.ap()[:, i * (DC // 2):(i + 1) * (DC // 2), ST:2 * ST],
                    )
                ck_t = tabs.tile([P, S], BF16, name="cosk")
                nc.sync.dma_start(ck_t[:], cos_k.ap())
                sk_t = tabs.tile([P, S], BF16, name="sink")
                nc.sync.dma_start(sk_t[:], sin_k.ap())
                # PE warm-up spin on pmat while the startup DMAs land: keeps
                # the PE's p-state ramp running so real work starts at speed
                warm = ppP.tile([P, ST], F32, tag="pk", name="warm")
                for _ in range(24):
                    nc.tensor.matmul(warm[:, 0:P], lhsT=pm_t[:], rhs=pm_t[:],
                                     start=True, stop=True)

                def xslice(xt, dc, s0=0, s1=ST):
                    n = DC // len(xt)
                    return xt[dc // n][:, dc % n, s0:s1]

                def wkslice(dc, jc):
                    return wk_sb[dc // (DC // 2)][:, dc % (DC // 2),
                                                  jc * P:(jc + 1) * P]

                pend = []  # deferred rope stage2
                xt_pre = {}  # prefetched x tiles, two ahead

                def flush_pend(n):
                    while len(pend) > n:
                        r, cs, ss, dst, tagp, uid = pend.pop(0)
                        pp = ppS.tile([P, ST], F32, tag="swp")
                        rope_stage2(r, pp[:], cs, ss, dst, ropeP, ST, tagp, uid)

                for st in range(NST):
                    if st == 0:
                        xt = xt0
                    elif st == 1:
                        xt = xt1
                    else:
                        xt = xt_pre[st]
                    if st + 2 < NST:
                        # prefetch two tiles ahead (ring WAR gates the xfer)
                        xt_pre[st + 2] = [
                            xin.tile([P, DC // 2, ST], BF16, tag=f"xh{i}",
                                     name=f"xt{st + 2}_{i}") for i in range(2)]
                        for i in range(2):
                            nc.sync.dma_start(
                                xt_pre[st + 2][i][:],
                                xT.ap()[:, i * (DC // 2):(i + 1) * (DC // 2),
                                        (st + 2) * ST:(st + 3) * ST],
                            )
                    sl = slice(st * ST, (st + 1) * ST)
                    if st == 0:
                        # interleave the two kv-head chains quarter-by-quarter
                        # so compute tracks the startup DMA arrivals
                        pks = [ppP.tile([P, ST], F32, tag="pk", name=f"pk0_{jc}")
                               for jc in range(KVL)]
                        for qp in range(4):
                            for jc in range(KVL):
                                for dc in range(qp * QC, (qp + 1) * QC):
                                    nc.tensor.matmul(
                                        pks[jc][:],
                                        lhsT=wkslice(dc, jc),
                                        rhs=xslice(xt, dc),
                                        start=(dc == 0),
                                        stop=(dc == DC - 1),
                                    )
                        for jc in range(KVL):
                            raw_sb = rope_stage1(pks[jc][:], ropeP, ST, "r",
                                                 f"k{st}_{jc}")
                            pend.append((raw_sb, ck_t[:, sl], sk_t[:, sl],
                                         kt[:, jc, sl], "r", f"k{st}_{jc}"))
                            flush_pend(1)
                    else:
                        for jc in range(KVL):
                            pk = ppP.tile([P, ST], F32, tag="pk")
                            for dc in range(DC):
                                nc.tensor.matmul(
                                    pk[:],
                                    lhsT=wkslice(dc, jc),
                                    rhs=xslice(xt, dc),
                                    start=(dc == 0),
                                    stop=(dc == DC - 1),
                                )
                            raw_sb = rope_stage1(pk[:], ropeP, ST, "r", f"k{st}_{jc}")
                            pend.append((raw_sb, ck_t[:, sl], sk_t[:, sl],
                                         kt[:, jc, sl], "r", f"k{st}_{jc}"))
                            flush_pend(1)
                    pv = ppV.tile([P, NST, KVJ], F32, tag="pv")
                    for si in range(NST):
                        for dc in range(DC):
                            nc.tensor.matmul(
                                pv[:, si, :],
                                lhsT=xslice(xt, dc, si * P, (si + 1) * P),
                                rhs=wv_sb[:, dc, :],
                                start=(dc == 0),
                                stop=(dc == DC - 1),
                            )
                    nc.scalar.copy(vt[:, st * NST:(st + 1) * NST, :], pv[:])
                    if st == NST - 1:
                        # issue Q-phase x re-stream DMAs. Only 2 up front
                        # (ring depth) so later DMAs aren't FIFO-blocked
                        # behind a WAR-held transfer. The last s-tile goes to
                        # its own long-lived pool: its Q projection is
                        # deferred into phase B to feed the PE there.
                        xtq = [[xin.tile([P, DC // 2, ST], BF16, tag=f"xh{i}",
                                         name=f"xtq{s}_{i}") for i in range(2)]
                               for s in range(NST)]

                        def dma_xtq(s):
                            for i in range(2):
                                nc.sync.dma_start(
                                    xtq[s][i][:],
                                    xT.ap()[:, i * (DC // 2):(i + 1) * (DC // 2),
                                            s * ST:(s + 1) * ST],
                                )

                        wq_sb = [wqp.tile([P, DC, P], BF16, name=f"wq{h}")
                                 for h in range(HL)]
                        cq_t = qtabs.tile([P, S], BF16, name="cosq")
                        sq_t = qtabs.tile([P, S], BF16, name="sinq")
                        dma_xtq(0)
                        for h in range(2):
                            nc.sync.dma_start(wq_sb[h][:], wq.ap()[h])
                        dma_xtq(1)
                        nc.sync.dma_start(cq_t[:], cos_q.ap())
                        nc.sync.dma_start(sq_t[:], sin_q.ap())
                        for h in range(2, HL):
                            nc.sync.dma_start(wq_sb[h][:], wq.ap()[h])

                # ---- Q projections, all 8 heads ----
                for st in range(NST if phases != "kv" else 0):
                    xt = xtq[st]
                    if st + 2 < NST:
                        dma_xtq(st + 2)
                    sl = slice(st * ST, (st + 1) * ST)
                    for h in range(HL):
                        pq = ppP.tile([P, ST], F32, tag="pk")
                        for dc in range(DC):
                            nc.tensor.matmul(
                                pq[:],
                                lhsT=wq_sb[h][:, dc, :],
                                rhs=xslice(xt, dc),
                                start=(dc == 0),
                                stop=(dc == DC - 1),
                            )
                        raw_sb = rope_stage1(pq[:], ropeP, ST, "r", f"q{st}_{h}")
                        pend.append((raw_sb, cq_t[:, sl], sq_t[:, sl],
                                     qh[h][:, sl], "r", f"q{st}_{h}"))
                        flush_pend(1)
                flush_pend(0)

            # ---- Phase B: attention + fused o_proj per q-tile ----
            with contextlib.ExitStack() as _stkB:
                _poolB = lambda name, bufs=1, **kw: _stkB.enter_context(
                    tc.tile_pool(name=name, bufs=bufs, **kw))
                ptp = _poolB("ptp", bufs=2)
                treep = _poolB("tree")
                attp = _poolB("attp", bufs=2)
                nrmp = _poolB("nrm", bufs=2)
                outp = _poolB("outp", bufs=3)
                ppSc = _poolB("ppSc", space="PSUM")
                ppAv = _poolB("ppAv", bufs=2, space="PSUM")
                ppO = _poolB("ppO", bufs=2, space="PSUM")
                GW = 4 * ST  # score/exp group: 4 key-chunks
                NG = S // GW  # 4 groups per unit

                def emit_den(u):
                    # den borrows a po-ring psum slot ([1, ST] of it)
                    den_t = ppO.tile([P, ST], F32, tag="po", name=f"den{u['id']}")
                    den = den_t[0:1, 0:ST]
                    nc.tensor.matmul(den, lhsT=ones_t[:], rhs=u["t4"][:],
                                     start=True, stop=True)
                    u["den"] = den

                def emit_normalize(u):
                    r_row = nrmp.tile([1, ST], F32, tag="rrow", name=f"rr{u['id']}")
                    nc.vector.reciprocal(r_row[:], u["den"])
                    rb = nrmp.tile([P, ST], F32, tag="rb", name=f"rb{u['id']}")
                    nc.gpsimd.partition_broadcast(rb[:], r_row[:])
                    nc.vector.tensor_tensor(u["att"][:], u["av"][:], rb[:], AL.mult)

                oq = []  # pending o_proj chains, emitted one per score group

                ocur = {}  # in-flight half-emitted o_proj chain

                def _emit_half(qt, att_set, qc, ot, po, h0, h1):
                    for h in range(h0, h1):
                        nc.tensor.matmul(
                            po[:],
                            lhsT=att_set[h][:, qc * P:(qc + 1) * P],
                            rhs=wo_sb[h][:, ot * ST:(ot + 1) * ST],
                            start=(h == 0),
                            stop=(h == HL - 1),
                        )

                def pop_ohalf():
                    # emit half an o_proj chain (4 of 8 accumulating matmuls)
                    if ocur:
                        qt, att_set, qc, ot, po = ocur.pop("c")
                        _emit_half(qt, att_set, qc, ot, po, HL // 2, HL)
                        out_t = outp.tile([P, ST], F32, tag="outt")
                        nc.vector.tensor_copy(out_t[:], po[:])
                        nc.sync.dma_start(
                            out.ap()[qt * ST + qc * P:qt * ST + (qc + 1) * P,
                                     ot * ST:(ot + 1) * ST],
                            out_t[:],
                        )
                    elif oq:
                        qt, att_set, qc, ot = oq.pop(0)
                        po = ppO.tile([P, ST], F32, tag="po")
                        _emit_half(qt, att_set, qc, ot, po, 0, HL // 2)
                        ocur["c"] = (qt, att_set, qc, ot, po)

                def pop_ochain(n=1):
                    for _ in range(2 * n):
                        pop_ohalf()

                for h in range(HL):
                    nc.sync.dma_start(wo_sb[h][:], wo.ap()[h])
                prev = None  # unit whose den/normalize is pending
                avq = []  # pending AV matmuls: (av_tile, kc, pt_tile, i, kv)
                qpend = []  # deferred rope stage2 for the st3 Q projections
                sl3 = slice((NST - 1) * ST, NST * ST)

                def flush_qpend():
                    while qpend:
                        r, ph = qpend.pop(0)
                        swp_t = ppO.tile([P, ST], F32, tag="po", name=f"swp3_{ph}")
                        rope_stage2(r, swp_t[:], cq_t[:, sl3], sq_t[:, sl3],
                                    qh[ph][:, sl3], ropeP, ST, "r", f"q3_{ph}")

                qp3 = {}  # in-flight deferred Q(st3) projection of this unit

                def emit_qproj3_part(h, part):
                    # Q projection of the last s-tile, deferred into phase B
                    # as PE fill work while ACT paces q-tile 0; two half-
                    # contractions so the po-ring slot isn't held too long.
                    if part == 0:
                        qp3["t"] = ppO.tile([P, ST], F32, tag="po", name=f"pq3_{h}")
                    for dc in range(part * DC // 2, (part + 1) * DC // 2):
                        nc.tensor.matmul(
                            qp3["t"][:],
                            lhsT=wq_sb[h][:, dc, :],
                            rhs=xslice(xtq[NST - 1], dc),
                            start=(dc == 0),
                            stop=(dc == DC - 1),
                        )
                    if part == 1:
                        raw_sb = rope_stage1(qp3["t"][:], ropeP, ST, "r", f"q3_{h}")
                        qpend.append((raw_sb, h))

                def flush_av(n):
                    while len(avq) > n:
                        avt, kc, ptt, i, fkv = avq.pop(0)
                        nc.tensor.matmul(
                            avt[:],
                            lhsT=vt[:, kc, fkv * P:(fkv + 1) * P],
                            rhs=ptt[:, i * ST:(i + 1) * ST],
                            start=(kc == 0),
                            stop=(kc == NKC - 1),
                        )

                for qt in range(NST if phases == "all" else 0):
                    qsl = slice(qt * ST, (qt + 1) * ST)
                    att_set = [attp.tile([P, ST], BF16, tag=f"att{h}",
                                         name=f"att{qt}_{h}") for h in range(HL)]
                    for h in range(HL):
                        uid = qt * HL + h
                        kv = h // (HL // KVL)
                        av = ppAv.tile([P, ST], F32, tag="av")
                        u = {"id": uid, "av": av, "att": att_set[h]}
                        pt = [None] * 8

                        run = None  # running sum of pt tiles (DVE, bf16 2x)
                        for kp in range(8):
                            sc_ps = ppSc.tile([P, 2 * ST], F32, tag="scores",
                                              bufs=2)
                            for i in range(2):
                                kc = kp * 2 + i
                                nc.tensor.matmul(
                                    sc_ps[:, i * ST:(i + 1) * ST],
                                    lhsT=kt[:, kv, kc * P:(kc + 1) * P],
                                    rhs=qh[h][:, qsl],
                                    start=True,
                                    stop=True,
                                )
                            if kp == 1:
                                flush_qpend()
                            if kp == 2 and prev is not None:
                                emit_den(prev)
                                emit_normalize(prev)
                                prev = None
                            pt[kp] = ptp.tile([P, 2 * ST], BF16, tag=f"pt{kp}",
                                              name=f"pt{uid}_{kp}")
                            nc.scalar.activation(pt[kp][:], sc_ps[:], AF.Exp)
                            for i in range(2):
                                avq.append((av, kp * 2 + i, pt[kp], i, kv))
                            flush_av(4)  # av trails scores by two kp
                            if kp > 0:
                                nxt = treep.tile([P, 2 * ST], BF16,
                                                 tag=f"rs{kp % 2}",
                                                 name=f"rs_{uid}_{kp}")
                                nc.vector.tensor_tensor(
                                    nxt[:], run[:] if kp > 1 else pt[0][:],
                                    pt[kp][:], AL.add
                                )
                                run = nxt
                            # half an o_proj chain of the previous q-tile
                            # every other kp
                            if kp % 2 == 1:
                                pop_ohalf()
                        t4 = treep.tile([P, ST], BF16, tag="t4", name=f"t4_{uid}")
                        nc.vector.tensor_tensor(
                            t4[:], run[:, 0:ST], run[:, ST:2 * ST], AL.add
                        )
                        u["t4"] = t4
                        prev = u
                    for qc in range(ST // P):
                        for ot in range(HID // ST):
                            oq.append((qt, att_set, qc, ot))
                # drain: pending AVs, last unit's den/normalize, last o_proj
                flush_av(0)
                if prev is not None:
                    emit_den(prev)
                    emit_normalize(prev)
                pop_ochain(len(oq))

    nc.compile()
    _CACHE["nc"] = nc
    return nc


def _host_inputs(x, Wq, Wk, Wv, Wo):
    """Build the 8 per-core input maps (numpy only)."""
    bf = ml_dtypes.bfloat16

    # rope tables: row j uses frequency j%64
    inv_ts = ROPE_THETA ** (-2.0 * np.arange(D // 2) / D)
    inv_full = np.concatenate([inv_ts, inv_ts])
    pos = np.arange(S, dtype=np.float64)
    ang = inv_full[:, None] * pos[None, :]
    cos_k = np.cos(ang).astype(bf)
    sin_k = np.sin(ang).astype(bf)
    scale = 1.0 / math.sqrt(D)
    cos_q = (np.cos(ang) * scale).astype(bf)
    sin_q = (np.sin(ang) * scale).astype(bf)

    pmat = np.zeros((P, P), np.float32)  # lhsT: swap[i] = -q[i+64] (i<64), +q[i-64]
    for i in range(64):
        pmat[i + 64, i] = -1.0
        pmat[i, i + 64] = 1.0
    pmat = pmat.astype(bf)
    ones = np.ones((P, 1), bf)

    in_maps = []
    for c in range(8):
        b, hg = c // 2, c % 2
        hsl = slice(hg * HL, (hg + 1) * HL)
        kvsl = slice(hg * KVL, (hg + 1) * KVL)
        xTb = np.ascontiguousarray(
            x[b].T.reshape(DC, P, S).transpose(1, 0, 2)
        ).astype(bf)  # [p, dc, s]
        wq_sw = np.ascontiguousarray(
            Wq[:, hsl, :].reshape(DC, P, HL, P).transpose(2, 1, 0, 3)
        ).astype(bf)  # [h, p, dc, j]
        wk_sw = np.ascontiguousarray(
            Wk[:, kvsl, :].reshape(DC, P, KVJ).transpose(1, 0, 2)
        ).astype(bf)  # [p, dc, j]
        wv_sw = np.ascontiguousarray(
            Wv[:, kvsl, :].reshape(DC, P, KVJ).transpose(1, 0, 2)
        ).astype(bf)
        wo_sw = np.ascontiguousarray(Wo[hsl]).astype(bf)  # [h, j(=d), o]
        in_maps.append(
            {
                "xT": xTb,
                "wq": wq_sw,
                "wk": wk_sw,
                "wv": wv_sw,
                "wo": wo_sw,
                "cos_q": cos_q,
                "sin_q": sin_q,
                "cos_k": cos_k,
                "sin_k": sin_k,
                "pmat": pmat,
                "ones": ones,
            }
        )
    return in_maps


def kernel(x, Wq, Wk, Wv, Wo, _trace=False):
    x, Wq, Wk, Wv, Wo = (np.asarray(a, dtype=np.float32) for a in (x, Wq, Wk, Wv, Wo))
    nc = build_nc()
    in_maps = _host_inputs(x, Wq, Wk, Wv, Wo)
    res = run_bass_kernel_spmd(nc, in_maps, core_ids=list(range(8)), trace=_trace)
    out = np.empty((B, S, HID), np.float32)
    for b in range(B):
        out[b] = res.results[2 * b]["out"]
        out[b] += res.results[2 * b + 1]["out"]
    if _trace:
        kernel.last_results = res
    return out


# revision 71
# speedup vs baseline: 1.3732x; 1.0103x over previous
"""Fused multi-head attention (RoPE + GQA + softmax + o_proj) on 8 Trainium2 cores.

Sharding: core c handles batch b = c//2 and head-group hg = c%2
(8 q-heads / 2 kv-heads), ALL 2048 queries.  Each core computes K/V for
only its kv heads, attention for its 8 q heads, and a PARTIAL o_proj
(contracted over its heads).  The host sums the two partial outputs per
batch (the "all-reduce after o_proj" of the tensor-parallel sharding).
Per-core matmul work is exactly 1/8 of the model total.

Everything runs in bf16 (1 cycle/row on the PE, same as f32r, but 2x on
DVE and half the DMA/SBUF), accumulating in f32 PSUM.

Pipelining (PE program order is execution order per engine):
 - rope's swap matmul for iteration u is emitted inside iteration u+1 so
   the PE never waits on the ACT psum->sbuf copy.
 - attention unit u = (qt, h): av matmuls trail the score matmuls by 2
   kp-steps so the ACT exp pipeline stays ahead of the PE.
 - the denominator matmul + normalize of unit u are emitted inside unit
   u+1 (tree latency hidden); den borrows a scores-ring PSUM slot.
 - o_proj of q-tile qt is emitted inside unit (qt+1, h0) so ACT/DVE of
   the next tile's units overlap its matmuls.

Per-core layouts (partition dim first):
  xT  [128, 16, S]   x[b]^T swizzled: partition=d%128, (dchunk, s)  bf16
  kt  [128, 2, S]    roped K, partition=d%128 of the kv head        bf16
  vt  [128, 16, 256] V, partition=s%128, (schunk, j of 2 kv heads)  bf16
  qh  [8][128, S]    roped Q per head, partition=d%128              bf16
  att [2][8][128, 512]  per q-tile: attention out, partition=j      bf16
"""

import contextlib
import os
import sys

sys.path.insert(0, "/opt/trn_rl_repo")

import math

import numpy as np
import ml_dtypes

import concourse.bass as bass
import concourse.mybir as mybir
import concourse.tile as tile
from concourse import bacc
from concourse.bass_utils import run_bass_kernel_spmd

P = 128
B, S, HID = 4, 2048, 2048
H, HKV, D = 16, 4, 128
HL = H // 2  # 8 q heads per core
KVL = HKV // 2  # 2 kv heads per core
DC = HID // P  # 16
KVJ = KVL * D  # 256
ST = 512  # s-tile for projections; also q-tile for attention
NST = S // ST  # 4
NKC = S // P  # 16 key chunks
ROPE_THETA = 10000.0

F32 = mybir.dt.float32
BF16 = mybir.dt.bfloat16
AL = mybir.AluOpType
AF = mybir.ActivationFunctionType

_CACHE = {}


def build_nc():
    if "nc" in _CACHE:
        return _CACHE["nc"]
    phases = os.environ.get("KERNEL_PHASES", "all")  # kv | kvq | all
    nc = bacc.Bacc("TRN2", target_bir_lowering=False)

    xT = nc.dram_tensor("xT", (P, DC, S), BF16, kind="ExternalInput")
    wq = nc.dram_tensor("wq", (HL, P, DC, P), BF16, kind="ExternalInput")
    wk = nc.dram_tensor("wk", (P, DC, KVJ), BF16, kind="ExternalInput")
    wv = nc.dram_tensor("wv", (P, DC, KVJ), BF16, kind="ExternalInput")
    wo = nc.dram_tensor("wo", (HL, P, HID), BF16, kind="ExternalInput")
    cos_q = nc.dram_tensor("cos_q", (P, S), BF16, kind="ExternalInput")
    sin_q = nc.dram_tensor("sin_q", (P, S), BF16, kind="ExternalInput")
    cos_k = nc.dram_tensor("cos_k", (P, S), BF16, kind="ExternalInput")
    sin_k = nc.dram_tensor("sin_k", (P, S), BF16, kind="ExternalInput")
    pmat = nc.dram_tensor("pmat", (P, P), BF16, kind="ExternalInput")
    ones = nc.dram_tensor("ones", (P, 1), BF16, kind="ExternalInput")
    out = nc.dram_tensor("out", (S, HID), F32, kind="ExternalOutput")

    with tile.TileContext(nc) as tc:
        with contextlib.ExitStack() as _stk:
            def _pool(name, bufs=1, **kw):
                return _stk.enter_context(tc.tile_pool(name=name, bufs=bufs, **kw))

            consts = _pool("consts")
            ktp = _pool("kt")
            vtp = _pool("vt")
            qhp = _pool("qh")
            wop = _pool("wop")
            wqp = _pool("wqp")
            qtabs = _pool("qtabs")
            xlast = _pool("xlast")
            ropeP = _pool("ropeP", bufs=2)
            pm_t = consts.tile([P, P], BF16)
            nc.sync.dma_start(pm_t[:], pmat.ap())
            ones_t = consts.tile([P, 1], BF16)
            nc.sync.dma_start(ones_t[:], ones.ap())
            kt = ktp.tile([P, KVL, S], BF16)
            vt = vtp.tile([P, NKC, KVJ], BF16)
            qh = [qhp.tile([P, S], BF16, name=f"qh{h}") for h in range(HL)]
            wo_sb = [wop.tile([P, HID], BF16, name=f"wo{h}") for h in range(HL)]

            def rope_stage1(raw_ps, work, w, tagp, u, dve=False):
                # psum -> sbuf bf16 copy of the raw projection (ACT, or DVE
                # when ACT is the pacing engine)
                raw_sb = work.tile([P, w], BF16, tag=f"{tagp}raw", name=f"rraw{u}")
                if dve:
                    nc.vector.tensor_copy(raw_sb[:], raw_ps)
                else:
                    nc.scalar.copy(raw_sb[:], raw_ps)
                return raw_sb

            def rope_stage2(raw_sb, swp_ps, cos_sl, sin_sl, dst, work, w, tagp, u,
                            dve=False):
                # PE: swap matmul; ACT/DVE: copy out; DVE: cos/sin mult-add
                nc.tensor.matmul(swp_ps, lhsT=pm_t[:], rhs=raw_sb[:],
                                 start=True, stop=True)
                swp_sb = work.tile([P, w], BF16, tag=f"{tagp}swp", name=f"rswp{u}")
                if dve:
                    nc.vector.tensor_copy(swp_sb[:], swp_ps)
                else:
                    nc.scalar.copy(swp_sb[:], swp_ps)
                ta = work.tile([P, w], BF16, tag=f"{tagp}a", name=f"ra{u}")
                nc.vector.tensor_tensor(ta[:], raw_sb[:], cos_sl, AL.mult)
                tb = work.tile([P, w], BF16, tag=f"{tagp}b", name=f"rb{u}")
                nc.vector.tensor_tensor(tb[:], swp_sb[:], sin_sl, AL.mult)
                nc.vector.tensor_tensor(dst, ta[:], tb[:], AL.add)

            # ---- Phase P: K/V projections then Q, one scope so the Q-phase
            # DMAs (x re-stream, wq, rope tables) are issued while KV computes.
            with contextlib.ExitStack() as _stkP:
                _poolP = lambda name, bufs=1, **kw: _stkP.enter_context(
                    tc.tile_pool(name=name, bufs=bufs, **kw))
                xin = _poolP("xin", bufs=2)
                tabs = _poolP("tabs")
                wkp = _poolP("wkp")
                ppP = _poolP("ppP", bufs=2, space="PSUM")
                ppS = _poolP("ppS", bufs=2, space="PSUM")
                ppV = _poolP("ppV", bufs=2, space="PSUM")
                # startup: quarter-granular first tiles so the PE can start
                # at ~3us and stream behind the DMA arrivals
                QC = DC // 4  # 4 dchunks per quarter
                wk_sb = [wkp.tile([P, DC // 2, KVJ], BF16, name=f"wk{i}")
                         for i in range(2)]
                nc.sync.dma_start(wk_sb[0][:], wk.ap()[:, 0:DC // 2, :])
                xt0 = [xin.tile([P, QC, ST], BF16, tag=f"xq{i}", name=f"xt0_{i}",
                                bufs=1)
                       for i in range(4)]
                nc.sync.dma_start(xt0[0][:], xT.ap()[:, 0:QC, 0:ST])
                nc.sync.dma_start(xt0[1][:], xT.ap()[:, QC:2 * QC, 0:ST])
                nc.sync.dma_start(wk_sb[1][:], wk.ap()[:, DC // 2:DC, :])
                nc.sync.dma_start(xt0[2][:], xT.ap()[:, 2 * QC:3 * QC, 0:ST])
                nc.sync.dma_start(xt0[3][:], xT.ap()[:, 3 * QC:4 * QC, 0:ST])
                wv_sb = wkp.tile([P, DC, KVJ], BF16, name="wv")
                nc.sync.dma_start(wv_sb[:], wv.ap())
                xt1 = [xin.tile([P, DC // 2, ST], BF16, tag=f"xh{i}",
                                name=f"xt1_{i}") for i in range(2)]
                for i in range(2):
                    nc.sync.dma_start(
                        xt1[i][:],
                        xT.ap()[:, i * (DC // 2):(i + 1) * (DC // 2), ST:2 * ST],
                    )
                ck_t = tabs.tile([P, S], BF16, name="cosk")
                nc.sync.dma_start(ck_t[:], cos_k.ap())
                sk_t = tabs.tile([P, S], BF16, name="sink")
                nc.sync.dma_start(sk_t[:], sin_k.ap())
                # PE warm-up spin on pmat while the startup DMAs land: keeps
                # the PE's p-state ramp running so real work starts at speed
                warm = ppP.tile([P, ST], F32, tag="pk", name="warm")
                for _ in range(24):
                    nc.tensor.matmul(warm[:, 0:P], lhsT=pm_t[:], rhs=pm_t[:],
                                     start=True, stop=True)

                def xslice(xt, dc, s0=0, s1=ST):
                    n = DC // len(xt)
                    return xt[dc // n][:, dc % n, s0:s1]

                def wkslice(dc, jc):
                    return wk_sb[dc // (DC // 2)][:, dc % (DC // 2),
                                                  jc * P:(jc + 1) * P]

                pend = []  # deferred rope stage2
                xt_pre = {}  # prefetched x tiles, two ahead

                def flush_pend(n):
                    while len(pend) > n:
                        r, cs, ss, dst, tagp, uid = pend.pop(0)
                        pp = ppS.tile([P, ST], F32, tag="swp")
                        rope_stage2(r, pp[:], cs, ss, dst, ropeP, ST, tagp, uid)

                for st in range(NST):
                    if st == 0:
                        xt = xt0
                    elif st == 1:
                        xt = xt1
                    else:
                        xt = xt_pre[st]
                    if st + 2 < NST:
                        # prefetch two tiles ahead (ring WAR gates the xfer)
                        xt_pre[st + 2] = [
                            xin.tile([P, DC // 2, ST], BF16, tag=f"xh{i}",
                                     name=f"xt{st + 2}_{i}") for i in range(2)]
                        for i in range(2):
                            nc.sync.dma_start(
                                xt_pre[st + 2][i][:],
                                xT.ap()[:, i * (DC // 2):(i + 1) * (DC // 2),
                                        (st + 2) * ST:(st + 3) * ST],
                            )
                    sl = slice(st * ST, (st + 1) * ST)
                    if st == 0:
                        # interleave the two kv-head chains quarter-by-quarter
                        # so compute tracks the startup DMA arrivals
                        pks = [ppP.tile([P, ST], F32, tag="pk", name=f"pk0_{jc}")
                               for jc in range(KVL)]
                        for qp in range(4):
                            for jc in range(KVL):
                                for dc in range(qp * QC, (qp + 1) * QC):
                                    nc.tensor.matmul(
                                        pks[jc][:],
                                        lhsT=wkslice(dc, jc),
                                        rhs=xslice(xt, dc),
                                        start=(dc == 0),
                                        stop=(dc == DC - 1),
                                    )
                        for jc in range(KVL):
                            raw_sb = rope_stage1(pks[jc][:], ropeP, ST, "r",
                                                 f"k{st}_{jc}")
                            pend.append((raw_sb, ck_t[:, sl], sk_t[:, sl],
                                         kt[:, jc, sl], "r", f"k{st}_{jc}"))
                            flush_pend(1)
                    else:
                        for jc in range(KVL):
                            pk = ppP.tile([P, ST], F32, tag="pk")
                            for dc in range(DC):
                                nc.tensor.matmul(
                                    pk[:],
                                    lhsT=wkslice(dc, jc),
                                    rhs=xslice(xt, dc),
                                    start=(dc == 0),
                                    stop=(dc == DC - 1),
                                )
                            raw_sb = rope_stage1(pk[:], ropeP, ST, "r", f"k{st}_{jc}")
                            pend.append((raw_sb, ck_t[:, sl], sk_t[:, sl],
                                         kt[:, jc, sl], "r", f"k{st}_{jc}"))
                            flush_pend(1)
                    pv = ppV.tile([P, NST, KVJ], F32, tag="pv")
                    for si in range(NST):
                        for dc in range(DC):
                            nc.tensor.matmul(
                                pv[:, si, :],
                                lhsT=xslice(xt, dc, si * P, (si + 1) * P),
                                rhs=wv_sb[:, dc, :],
                                start=(dc == 0),
                                stop=(dc == DC - 1),
                            )
                    nc.scalar.copy(vt[:, st * NST:(st + 1) * NST, :], pv[:])
                    if st == NST - 1:
                        # issue Q-phase x re-stream DMAs. Only 2 up front
                        # (ring depth) so later DMAs aren't FIFO-blocked
                        # behind a WAR-held transfer. The last s-tile goes to
                        # its own long-lived pool: its Q projection is
                        # deferred into phase B to feed the PE there.
                        xtq = [[(xlast if s == NST - 1 else xin).tile(
                                    [P, DC // 2, ST], BF16,
                                    tag=(f"xh{i}" if s < NST - 1 else f"xl{i}"),
                                    name=f"xtq{s}_{i}") for i in range(2)]
                               for s in range(NST)]

                        def dma_xtq(s):
                            for i in range(2):
                                nc.sync.dma_start(
                                    xtq[s][i][:],
                                    xT.ap()[:, i * (DC // 2):(i + 1) * (DC // 2),
                                            s * ST:(s + 1) * ST],
                                )

                        wq_sb = [wqp.tile([P, DC, P], BF16, name=f"wq{h}")
                                 for h in range(HL)]
                        cq_t = qtabs.tile([P, S], BF16, name="cosq")
                        sq_t = qtabs.tile([P, S], BF16, name="sinq")
                        dma_xtq(0)
                        for h in range(2):
                            nc.sync.dma_start(wq_sb[h][:], wq.ap()[h])
                        dma_xtq(1)
                        nc.sync.dma_start(cq_t[:], cos_q.ap())
                        nc.sync.dma_start(sq_t[:], sin_q.ap())
                        for h in range(2, HL):
                            nc.sync.dma_start(wq_sb[h][:], wq.ap()[h])

                # ---- Q projections, s-tiles 0..NST-2 (st3 deferred to B) ----
                for st in range(NST - 1 if phases != "kv" else 0):
                    xt = xtq[st]
                    if st + 2 < NST:
                        dma_xtq(st + 2)
                    sl = slice(st * ST, (st + 1) * ST)
                    for h in range(HL):
                        pq = ppP.tile([P, ST], F32, tag="pk")
                        for dc in range(DC):
                            nc.tensor.matmul(
                                pq[:],
                                lhsT=wq_sb[h][:, dc, :],
                                rhs=xslice(xt, dc),
                                start=(dc == 0),
                                stop=(dc == DC - 1),
                            )
                        raw_sb = rope_stage1(pq[:], ropeP, ST, "r", f"q{st}_{h}")
                        pend.append((raw_sb, cq_t[:, sl], sq_t[:, sl],
                                     qh[h][:, sl], "r", f"q{st}_{h}"))
                        flush_pend(1)
                flush_pend(0)

            # ---- Phase B: attention + fused o_proj per q-tile ----
            with contextlib.ExitStack() as _stkB:
                _poolB = lambda name, bufs=1, **kw: _stkB.enter_context(
                    tc.tile_pool(name=name, bufs=bufs, **kw))
                ptp = _poolB("ptp", bufs=2)
                treep = _poolB("tree")
                attp = _poolB("attp", bufs=2)
                nrmp = _poolB("nrm", bufs=2)
                outp = _poolB("outp", bufs=3)
                ppSc = _poolB("ppSc", space="PSUM")
                ppAv = _poolB("ppAv", bufs=2, space="PSUM")
                ppO = _poolB("ppO", bufs=2, space="PSUM")
                GW = 4 * ST  # score/exp group: 4 key-chunks
                NG = S // GW  # 4 groups per unit

                def emit_den(u):
                    # den borrows a po-ring psum slot ([1, ST] of it)
                    den_t = ppO.tile([P, ST], F32, tag="po", name=f"den{u['id']}")
                    den = den_t[0:1, 0:ST]
                    nc.tensor.matmul(den, lhsT=ones_t[:], rhs=u["t4"][:],
                                     start=True, stop=True)
                    u["den"] = den

                def emit_normalize(u):
                    r_row = nrmp.tile([1, ST], F32, tag="rrow", name=f"rr{u['id']}")
                    nc.vector.reciprocal(r_row[:], u["den"])
                    rb = nrmp.tile([P, ST], F32, tag="rb", name=f"rb{u['id']}")
                    nc.gpsimd.partition_broadcast(rb[:], r_row[:])
                    nc.vector.tensor_tensor(u["att"][:], u["av"][:], rb[:], AL.mult)

                oq = []  # pending o_proj chains, emitted one per score group

                ocur = {}  # in-flight half-emitted o_proj chain

                def _emit_half(qt, att_set, qc, ot, po, h0, h1):
                    for h in range(h0, h1):
                        nc.tensor.matmul(
                            po[:],
                            lhsT=att_set[h][:, qc * P:(qc + 1) * P],
                            rhs=wo_sb[h][:, ot * ST:(ot + 1) * ST],
                            start=(h == 0),
                            stop=(h == HL - 1),
                        )

                def pop_ohalf():
                    # emit half an o_proj chain (4 of 8 accumulating matmuls)
                    if ocur:
                        qt, att_set, qc, ot, po = ocur.pop("c")
                        _emit_half(qt, att_set, qc, ot, po, HL // 2, HL)
                        out_t = outp.tile([P, ST], F32, tag="outt")
                        nc.vector.tensor_copy(out_t[:], po[:])
                        nc.sync.dma_start(
                            out.ap()[qt * ST + qc * P:qt * ST + (qc + 1) * P,
                                     ot * ST:(ot + 1) * ST],
                            out_t[:],
                        )
                    elif oq:
                        qt, att_set, qc, ot = oq.pop(0)
                        po = ppO.tile([P, ST], F32, tag="po")
                        _emit_half(qt, att_set, qc, ot, po, 0, HL // 2)
                        ocur["c"] = (qt, att_set, qc, ot, po)

                def pop_ochain(n=1):
                    for _ in range(2 * n):
                        pop_ohalf()

                for h in range(HL):
                    nc.sync.dma_start(wo_sb[h][:], wo.ap()[h])
                prev = None  # unit whose den/normalize is pending
                avq = []  # pending AV matmuls: (av_tile, kc, pt_tile, i, kv)
                qpend = []  # deferred rope stage2 for the st3 Q projections
                sl3 = slice((NST - 1) * ST, NST * ST)

                def flush_qpend():
                    while qpend:
                        r, ph = qpend.pop(0)
                        swp_t = ppO.tile([P, ST], F32, tag="po", name=f"swp3_{ph}")
                        rope_stage2(r, swp_t[:], cq_t[:, sl3], sq_t[:, sl3],
                                    qh[ph][:, sl3], ropeP, ST, "r", f"q3_{ph}",
                                    dve=True)

                qp3 = {}  # in-flight deferred Q(st3) projection of this unit

                def emit_qproj3_part(h, part):
                    # Q projection of the last s-tile, deferred into phase B
                    # as PE fill work while ACT paces q-tile 0; two half-
                    # contractions so the po-ring slot isn't held too long.
                    if part == 0:
                        qp3["t"] = ppO.tile([P, ST], F32, tag="po", name=f"pq3_{h}")
                    for dc in range(part * DC // 2, (part + 1) * DC // 2):
                        nc.tensor.matmul(
                            qp3["t"][:],
                            lhsT=wq_sb[h][:, dc, :],
                            rhs=xslice(xtq[NST - 1], dc),
                            start=(dc == 0),
                            stop=(dc == DC - 1),
                        )
                    if part == 1:
                        raw_sb = rope_stage1(qp3["t"][:], ropeP, ST, "r", f"q3_{h}",
                                             dve=True)
                        qpend.append((raw_sb, h))

                def flush_av(n):
                    while len(avq) > n:
                        avt, kc, ptt, i, fkv = avq.pop(0)
                        nc.tensor.matmul(
                            avt[:],
                            lhsT=vt[:, kc, fkv * P:(fkv + 1) * P],
                            rhs=ptt[:, i * ST:(i + 1) * ST],
                            start=(kc == 0),
                            stop=(kc == NKC - 1),
                        )

                for qt in range(NST if phases == "all" else 0):
                    qsl = slice(qt * ST, (qt + 1) * ST)
                    att_set = [attp.tile([P, ST], BF16, tag=f"att{h}",
                                         name=f"att{qt}_{h}") for h in range(HL)]
                    for h in range(HL):
                        uid = qt * HL + h
                        kv = h // (HL // KVL)
                        av = ppAv.tile([P, ST], F32, tag="av")
                        u = {"id": uid, "av": av, "att": att_set[h]}
                        pt = [None] * 8

                        run = None  # running sum of pt tiles (DVE, bf16 2x)
                        for kp in range(8):
                            sc_ps = ppSc.tile([P, 2 * ST], F32, tag="scores",
                                              bufs=2)
                            for i in range(2):
                                kc = kp * 2 + i
                                nc.tensor.matmul(
                                    sc_ps[:, i * ST:(i + 1) * ST],
                                    lhsT=kt[:, kv, kc * P:(kc + 1) * P],
                                    rhs=qh[h][:, qsl],
                                    start=True,
                                    stop=True,
                                )
                            if kp == 1:
                                flush_qpend()
                            if qt == 0 and phases == "all" and kp in (3, 5):
                                emit_qproj3_part(h, (kp - 3) // 2)
                            if kp == 2 and prev is not None:
                                emit_den(prev)
                                emit_normalize(prev)
                                prev = None
                            pt[kp] = ptp.tile([P, 2 * ST], BF16, tag=f"pt{kp}",
                                              name=f"pt{uid}_{kp}")
                            nc.scalar.activation(pt[kp][:], sc_ps[:], AF.Exp)
                            for i in range(2):
                                avq.append((av, kp * 2 + i, pt[kp], i, kv))
                            flush_av(4)  # av trails scores by two kp
                            if kp > 0:
                                nxt = treep.tile([P, 2 * ST], BF16,
                                                 tag=f"rs{kp % 2}",
                                                 name=f"rs_{uid}_{kp}")
                                nc.vector.tensor_tensor(
                                    nxt[:], run[:] if kp > 1 else pt[0][:],
                                    pt[kp][:], AL.add
                                )
                                run = nxt
                            # half an o_proj chain of the previous q-tile
                            # every other kp
                            if kp % 2 == 1:
                                pop_ohalf()
                        t4 = treep.tile([P, ST], BF16, tag="t4", name=f"t4_{uid}")
                        nc.vector.tensor_tensor(
                            t4[:], run[:, 0:ST], run[:, ST:2 * ST], AL.add
                        )
                        u["t4"] = t4
                        prev = u
                    for qc in range(ST // P):
                        for ot in range(HID // ST):
                            oq.append((qt, att_set, qc, ot))
                # drain: pending AVs, last unit's den/normalize, last o_proj
                flush_av(0)
                if prev is not None:
                    emit_den(prev)
                    emit_normalize(prev)
                pop_ochain(len(oq))

    nc.compile()
    _CACHE["nc"] = nc
    return nc


def _host_inputs(x, Wq, Wk, Wv, Wo):
    """Build the 8 per-core input maps (numpy only)."""
    bf = ml_dtypes.bfloat16

    # rope tables: row j uses frequency j%64
    inv_ts = ROPE_THETA ** (-2.0 * np.arange(D // 2) / D)
    inv_full = np.concatenate([inv_ts, inv_ts])
    pos = np.arange(S, dtype=np.float64)
    ang = inv_full[:, None] * pos[None, :]
    cos_k = np.cos(ang).astype(bf)
    sin_k = np.sin(ang).astype(bf)
    scale = 1.0 / math.sqrt(D)
    cos_q = (np.cos(ang) * scale).astype(bf)
    sin_q = (np.sin(ang) * scale).astype(bf)

    pmat = np.zeros((P, P), np.float32)  # lhsT: swap[i] = -q[i+64] (i<64), +q[i-64]
    for i in range(64):
        pmat[i + 64, i] = -1.0
        pmat[i, i + 64] = 1.0
    pmat = pmat.astype(bf)
    ones = np.ones((P, 1), bf)

    in_maps = []
    for c in range(8):
        b, hg = c // 2, c % 2
        hsl = slice(hg * HL, (hg + 1) * HL)
        kvsl = slice(hg * KVL, (hg + 1) * KVL)
        xTb = np.ascontiguousarray(
            x[b].T.reshape(DC, P, S).transpose(1, 0, 2)
        ).astype(bf)  # [p, dc, s]
        wq_sw = np.ascontiguousarray(
            Wq[:, hsl, :].reshape(DC, P, HL, P).transpose(2, 1, 0, 3)
        ).astype(bf)  # [h, p, dc, j]
        wk_sw = np.ascontiguousarray(
            Wk[:, kvsl, :].reshape(DC, P, KVJ).transpose(1, 0, 2)
        ).astype(bf)  # [p, dc, j]
        wv_sw = np.ascontiguousarray(
            Wv[:, kvsl, :].reshape(DC, P, KVJ).transpose(1, 0, 2)
        ).astype(bf)
        wo_sw = np.ascontiguousarray(Wo[hsl]).astype(bf)  # [h, j(=d), o]
        in_maps.append(
            {
                "xT": xTb,
                "wq": wq_sw,
                "wk": wk_sw,
                "wv": wv_sw,
                "wo": wo_sw,
                "cos_q": cos_q,
                "sin_q": sin_q,
                "cos_k": cos_k,
                "sin_k": sin_k,
                "pmat": pmat,
                "ones": ones,
            }
        )
    return in_maps


def kernel(x, Wq, Wk, Wv, Wo, _trace=False):
    x, Wq, Wk, Wv, Wo = (np.asarray(a, dtype=np.float32) for a in (x, Wq, Wk, Wv, Wo))
    nc = build_nc()
    in_maps = _host_inputs(x, Wq, Wk, Wv, Wo)
    res = run_bass_kernel_spmd(nc, in_maps, core_ids=list(range(8)), trace=_trace)
    out = np.empty((B, S, HID), np.float32)
    for b in range(B):
        out[b] = res.results[2 * b]["out"]
        out[b] += res.results[2 * b + 1]["out"]
    if _trace:
        kernel.last_results = res
    return out


# revision 75
# speedup vs baseline: 1.3956x; 1.0163x over previous
"""Fused multi-head attention (RoPE + GQA + softmax + o_proj) on 8 Trainium2 cores.

Sharding: core c handles batch b = c//2 and head-group hg = c%2
(8 q-heads / 2 kv-heads), ALL 2048 queries.  Each core computes K/V for
only its kv heads, attention for its 8 q heads, and a PARTIAL o_proj
(contracted over its heads).  The host sums the two partial outputs per
batch (the "all-reduce after o_proj" of the tensor-parallel sharding).
Per-core matmul work is exactly 1/8 of the model total.

Everything runs in bf16 (1 cycle/row on the PE, same as f32r, but 2x on
DVE and half the DMA/SBUF), accumulating in f32 PSUM.

Pipelining (PE program order is execution order per engine):
 - rope's swap matmul for iteration u is emitted inside iteration u+1 so
   the PE never waits on the ACT psum->sbuf copy.
 - attention unit u = (qt, h): av matmuls trail the score matmuls by 2
   kp-steps so the ACT exp pipeline stays ahead of the PE.
 - the denominator matmul + normalize of unit u are emitted inside unit
   u+1 (tree latency hidden); den borrows a scores-ring PSUM slot.
 - o_proj of q-tile qt is emitted inside unit (qt+1, h0) so ACT/DVE of
   the next tile's units overlap its matmuls.

Per-core layouts (partition dim first):
  xT  [128, 16, S]   x[b]^T swizzled: partition=d%128, (dchunk, s)  bf16
  kt  [128, 2, S]    roped K, partition=d%128 of the kv head        bf16
  vt  [128, 16, 256] V, partition=s%128, (schunk, j of 2 kv heads)  bf16
  qh  [8][128, S]    roped Q per head, partition=d%128              bf16
  att [2][8][128, 512]  per q-tile: attention out, partition=j      bf16
"""

import contextlib
import os
import sys

sys.path.insert(0, "/opt/trn_rl_repo")

import math

import numpy as np
import ml_dtypes

import concourse.bass as bass
import concourse.mybir as mybir
import concourse.tile as tile
from concourse import bacc
from concourse.bass_utils import run_bass_kernel_spmd

P = 128
B, S, HID = 4, 2048, 2048
H, HKV, D = 16, 4, 128
HL = H // 2  # 8 q heads per core
KVL = HKV // 2  # 2 kv heads per core
DC = HID // P  # 16
KVJ = KVL * D  # 256
ST = 512  # s-tile for projections; also q-tile for attention
NST = S // ST  # 4
NKC = S // P  # 16 key chunks
ROPE_THETA = 10000.0

F32 = mybir.dt.float32
BF16 = mybir.dt.bfloat16
AL = mybir.AluOpType
AF = mybir.ActivationFunctionType

_CACHE = {}


def build_nc():
    if "nc" in _CACHE:
        return _CACHE["nc"]
    phases = os.environ.get("KERNEL_PHASES", "all")  # kv | kvq | all
    nc = bacc.Bacc("TRN2", target_bir_lowering=False)

    xT = nc.dram_tensor("xT", (P, DC, S), BF16, kind="ExternalInput")
    wq = nc.dram_tensor("wq", (HL, P, DC, P), BF16, kind="ExternalInput")
    wk = nc.dram_tensor("wk", (P, DC, KVJ), BF16, kind="ExternalInput")
    wv = nc.dram_tensor("wv", (P, DC, KVJ), BF16, kind="ExternalInput")
    wo = nc.dram_tensor("wo", (HL, P, HID), BF16, kind="ExternalInput")
    cos_q = nc.dram_tensor("cos_q", (P, S), BF16, kind="ExternalInput")
    sin_q = nc.dram_tensor("sin_q", (P, S), BF16, kind="ExternalInput")
    cos_k = nc.dram_tensor("cos_k", (P, S), BF16, kind="ExternalInput")
    sin_k = nc.dram_tensor("sin_k", (P, S), BF16, kind="ExternalInput")
    pmat = nc.dram_tensor("pmat", (P, P), BF16, kind="ExternalInput")
    ones = nc.dram_tensor("ones", (P, 1), BF16, kind="ExternalInput")
    out = nc.dram_tensor("out", (S, HID), F32, kind="ExternalOutput")

    with tile.TileContext(nc) as tc:
        with contextlib.ExitStack() as _stk:
            def _pool(name, bufs=1, **kw):
                return _stk.enter_context(tc.tile_pool(name=name, bufs=bufs, **kw))

            consts = _pool("consts")
            ktp = _pool("kt")
            vtp = _pool("vt")
            qhp = _pool("qh")
            wop = _pool("wop")
            wqp = _pool("wqp")
            qtabs = _pool("qtabs")
            xlast = _pool("xlast")
            ropeP = _pool("ropeP", bufs=2)
            pm_t = consts.tile([P, P], BF16)
            nc.sync.dma_start(pm_t[:], pmat.ap())
            ones_t = consts.tile([P, 1], BF16)
            nc.sync.dma_start(ones_t[:], ones.ap())
            kt = ktp.tile([P, KVL, S], BF16)
            vt = vtp.tile([P, NKC, KVJ], BF16)
            qh = [qhp.tile([P, S], BF16, name=f"qh{h}") for h in range(HL)]
            wo_sb = [wop.tile([P, HID], BF16, name=f"wo{h}") for h in range(HL)]

            def rope_stage1(raw_ps, work, w, tagp, u, dve=False):
                # psum -> sbuf bf16 copy of the raw projection (ACT, or DVE
                # when ACT is the pacing engine)
                raw_sb = work.tile([P, w], BF16, tag=f"{tagp}raw", name=f"rraw{u}")
                if dve:
                    nc.vector.tensor_copy(raw_sb[:], raw_ps)
                else:
                    nc.scalar.copy(raw_sb[:], raw_ps)
                return raw_sb

            def rope_stage2(raw_sb, swp_ps, cos_sl, sin_sl, dst, work, w, tagp, u,
                            dve=False):
                # PE: swap matmul; ACT/DVE: copy out; DVE: cos/sin mult-add
                nc.tensor.matmul(swp_ps, lhsT=pm_t[:], rhs=raw_sb[:],
                                 start=True, stop=True)
                swp_sb = work.tile([P, w], BF16, tag=f"{tagp}swp", name=f"rswp{u}")
                if dve:
                    nc.vector.tensor_copy(swp_sb[:], swp_ps)
                else:
                    nc.scalar.copy(swp_sb[:], swp_ps)
                ta = work.tile([P, w], BF16, tag=f"{tagp}a", name=f"ra{u}")
                nc.vector.tensor_tensor(ta[:], raw_sb[:], cos_sl, AL.mult)
                tb = work.tile([P, w], BF16, tag=f"{tagp}b", name=f"rb{u}")
                nc.vector.tensor_tensor(tb[:], swp_sb[:], sin_sl, AL.mult)
                nc.vector.tensor_tensor(dst, ta[:], tb[:], AL.add)

            # ---- Phase P: K/V projections then Q, one scope so the Q-phase
            # DMAs (x re-stream, wq, rope tables) are issued while KV computes.
            with contextlib.ExitStack() as _stkP:
                _poolP = lambda name, bufs=1, **kw: _stkP.enter_context(
                    tc.tile_pool(name=name, bufs=bufs, **kw))
                xin = _poolP("xin", bufs=2)
                tabs = _poolP("tabs")
                wkp = _poolP("wkp")
                ppP = _poolP("ppP", bufs=2, space="PSUM")
                ppS = _poolP("ppS", bufs=2, space="PSUM")
                ppV = _poolP("ppV", bufs=2, space="PSUM")
                # startup: quarter-granular first tiles so the PE can start
                # at ~3us and stream behind the DMA arrivals
                wk_sb = [wkp.tile([P, DC // 2, KVJ], BF16, name=f"wk{i}")
                         for i in range(2)]
                nc.sync.dma_start(wk_sb[0][:], wk.ap()[:, 0:DC // 2, :])
                xt0 = [xin.tile([P, DC // 2, ST], BF16, tag=f"xh{i}",
                                name=f"xt0_{i}") for i in range(2)]
                nc.sync.dma_start(xt0[0][:], xT.ap()[:, 0:DC // 2, 0:ST])
                nc.sync.dma_start(wk_sb[1][:], wk.ap()[:, DC // 2:DC, :])
                nc.sync.dma_start(xt0[1][:], xT.ap()[:, DC // 2:DC, 0:ST])
                wv_sb = wkp.tile([P, DC, KVJ], BF16, name="wv")
                nc.sync.dma_start(wv_sb[:], wv.ap())
                xt1 = [xin.tile([P, DC // 2, ST], BF16, tag=f"xh{i}",
                                name=f"xt1_{i}") for i in range(2)]
                for i in range(2):
                    nc.sync.dma_start(
                        xt1[i][:],
                        xT.ap()[:, i * (DC // 2):(i + 1) * (DC // 2), ST:2 * ST],
                    )
                ck_t = tabs.tile([P, S], BF16, name="cosk")
                nc.sync.dma_start(ck_t[:], cos_k.ap())
                sk_t = tabs.tile([P, S], BF16, name="sink")
                nc.sync.dma_start(sk_t[:], sin_k.ap())
                # PE warm-up spin on pmat while the startup DMAs land: keeps
                # the PE's p-state ramp running so real work starts at speed
                warm = ppP.tile([P, ST], F32, tag="pk", name="warm")
                for _ in range(24):
                    nc.tensor.matmul(warm[:, 0:P], lhsT=pm_t[:], rhs=pm_t[:],
                                     start=True, stop=True)

                def xslice(xt, dc, s0=0, s1=ST):
                    n = DC // len(xt)
                    return xt[dc // n][:, dc % n, s0:s1]

                def wkslice(dc, jc):
                    return wk_sb[dc // (DC // 2)][:, dc % (DC // 2),
                                                  jc * P:(jc + 1) * P]

                pend = []  # deferred rope stage2
                xt_pre = {}  # prefetched x tiles, two ahead

                def flush_pend(n):
                    while len(pend) > n:
                        r, cs, ss, dst, tagp, uid = pend.pop(0)
                        pp = ppS.tile([P, ST], F32, tag="swp")
                        rope_stage2(r, pp[:], cs, ss, dst, ropeP, ST, tagp, uid)

                for st in range(NST):
                    if st == 0:
                        xt = xt0
                    elif st == 1:
                        xt = xt1
                    else:
                        xt = xt_pre[st]
                    if st + 2 < NST:
                        # prefetch two tiles ahead (ring WAR gates the xfer)
                        xt_pre[st + 2] = [
                            xin.tile([P, DC // 2, ST], BF16, tag=f"xh{i}",
                                     name=f"xt{st + 2}_{i}") for i in range(2)]
                        for i in range(2):
                            nc.sync.dma_start(
                                xt_pre[st + 2][i][:],
                                xT.ap()[:, i * (DC // 2):(i + 1) * (DC // 2),
                                        (st + 2) * ST:(st + 3) * ST],
                            )
                    sl = slice(st * ST, (st + 1) * ST)
                    if True:
                        for jc in range(KVL):
                            pk = ppP.tile([P, ST], F32, tag="pk")
                            for dc in range(DC):
                                nc.tensor.matmul(
                                    pk[:],
                                    lhsT=wkslice(dc, jc),
                                    rhs=xslice(xt, dc),
                                    start=(dc == 0),
                                    stop=(dc == DC - 1),
                                )
                            raw_sb = rope_stage1(pk[:], ropeP, ST, "r", f"k{st}_{jc}")
                            pend.append((raw_sb, ck_t[:, sl], sk_t[:, sl],
                                         kt[:, jc, sl], "r", f"k{st}_{jc}"))
                            flush_pend(1)
                    pv = ppV.tile([P, NST, KVJ], F32, tag="pv")
                    for si in range(NST):
                        for dc in range(DC):
                            nc.tensor.matmul(
                                pv[:, si, :],
                                lhsT=xslice(xt, dc, si * P, (si + 1) * P),
                                rhs=wv_sb[:, dc, :],
                                start=(dc == 0),
                                stop=(dc == DC - 1),
                            )
                    nc.scalar.copy(vt[:, st * NST:(st + 1) * NST, :], pv[:])
                    if st == NST - 1:
                        # issue Q-phase x re-stream DMAs. Only 2 up front
                        # (ring depth) so later DMAs aren't FIFO-blocked
                        # behind a WAR-held transfer. The last s-tile goes to
                        # its own long-lived pool: its Q projection is
                        # deferred into phase B to feed the PE there.
                        xtq = [[(xlast if s == NST - 1 else xin).tile(
                                    [P, DC // 2, ST], BF16,
                                    tag=(f"xh{i}" if s < NST - 1 else f"xl{i}"),
                                    name=f"xtq{s}_{i}") for i in range(2)]
                               for s in range(NST)]

                        def dma_xtq(s):
                            for i in range(2):
                                nc.sync.dma_start(
                                    xtq[s][i][:],
                                    xT.ap()[:, i * (DC // 2):(i + 1) * (DC // 2),
                                            s * ST:(s + 1) * ST],
                                )

                        wq_sb = [wqp.tile([P, DC, P], BF16, name=f"wq{h}")
                                 for h in range(HL)]
                        cq_t = qtabs.tile([P, S], BF16, name="cosq")
                        sq_t = qtabs.tile([P, S], BF16, name="sinq")
                        dma_xtq(0)
                        for h in range(2):
                            nc.sync.dma_start(wq_sb[h][:], wq.ap()[h])
                        dma_xtq(1)
                        nc.sync.dma_start(cq_t[:], cos_q.ap())
                        nc.sync.dma_start(sq_t[:], sin_q.ap())
                        for h in range(2, HL):
                            nc.sync.dma_start(wq_sb[h][:], wq.ap()[h])

                # ---- Q projections, s-tiles 0..NST-2 (st3 deferred to B) ----
                for st in range(NST - 1 if phases != "kv" else 0):
                    xt = xtq[st]
                    if st + 2 < NST:
                        dma_xtq(st + 2)
                    sl = slice(st * ST, (st + 1) * ST)
                    for h in range(HL):
                        pq = ppP.tile([P, ST], F32, tag="pk")
                        for dc in range(DC):
                            nc.tensor.matmul(
                                pq[:],
                                lhsT=wq_sb[h][:, dc, :],
                                rhs=xslice(xt, dc),
                                start=(dc == 0),
                                stop=(dc == DC - 1),
                            )
                        raw_sb = rope_stage1(pq[:], ropeP, ST, "r", f"q{st}_{h}")
                        pend.append((raw_sb, cq_t[:, sl], sq_t[:, sl],
                                     qh[h][:, sl], "r", f"q{st}_{h}"))
                        flush_pend(1)
                flush_pend(0)

            # ---- Phase B: attention + fused o_proj per q-tile ----
            with contextlib.ExitStack() as _stkB:
                _poolB = lambda name, bufs=1, **kw: _stkB.enter_context(
                    tc.tile_pool(name=name, bufs=bufs, **kw))
                ptp = _poolB("ptp", bufs=1)
                treep = _poolB("tree")
                attp = _poolB("attp", bufs=2)
                nrmp = _poolB("nrm", bufs=2)
                outp = _poolB("outp", bufs=3)
                ppSc = _poolB("ppSc", space="PSUM")
                ppAv = _poolB("ppAv", bufs=2, space="PSUM")
                ppO = _poolB("ppO", bufs=2, space="PSUM")
                GW = 4 * ST  # score/exp group: 4 key-chunks
                NG = S // GW  # 4 groups per unit

                def emit_den(u):
                    # den borrows a po-ring psum slot ([1, ST] of it)
                    den_t = ppO.tile([P, ST], F32, tag="po", name=f"den{u['id']}")
                    den = den_t[0:1, 0:ST]
                    nc.tensor.matmul(den, lhsT=ones_t[:], rhs=u["t4"][:],
                                     start=True, stop=True)
                    u["den"] = den

                def emit_normalize(u):
                    r_row = nrmp.tile([1, ST], F32, tag="rrow", name=f"rr{u['id']}")
                    nc.vector.reciprocal(r_row[:], u["den"])
                    rb = nrmp.tile([P, ST], F32, tag="rb", name=f"rb{u['id']}")
                    nc.gpsimd.partition_broadcast(rb[:], r_row[:])
                    nc.vector.tensor_tensor(u["att"][:], u["av"][:], rb[:], AL.mult)

                oq = []  # pending o_proj chains, emitted one per score group

                ocur = {}  # in-flight half-emitted o_proj chain

                def _emit_half(qt, att_set, qc, ot, po, h0, h1):
                    for h in range(h0, h1):
                        nc.tensor.matmul(
                            po[:],
                            lhsT=att_set[h][:, qc * P:(qc + 1) * P],
                            rhs=wo_sb[h][:, ot * ST:(ot + 1) * ST],
                            start=(h == 0),
                            stop=(h == HL - 1),
                        )

                def pop_ohalf():
                    # emit half an o_proj chain (4 of 8 accumulating matmuls)
                    if ocur:
                        qt, att_set, qc, ot, po = ocur.pop("c")
                        _emit_half(qt, att_set, qc, ot, po, HL // 2, HL)
                        out_t = outp.tile([P, ST], F32, tag="outt")
                        nc.vector.tensor_copy(out_t[:], po[:])
                        nc.sync.dma_start(
                            out.ap()[qt * ST + qc * P:qt * ST + (qc + 1) * P,
                                     ot * ST:(ot + 1) * ST],
                            out_t[:],
                        )
                    elif oq:
                        qt, att_set, qc, ot = oq.pop(0)
                        po = ppO.tile([P, ST], F32, tag="po")
                        _emit_half(qt, att_set, qc, ot, po, 0, HL // 2)
                        ocur["c"] = (qt, att_set, qc, ot, po)

                def pop_ochain(n=1):
                    for _ in range(2 * n):
                        pop_ohalf()

                for h in range(HL):
                    nc.sync.dma_start(wo_sb[h][:], wo.ap()[h])
                prev = None  # unit whose den/normalize is pending
                avq = []  # pending AV matmuls: (av_tile, kc, pt_tile, i, kv)
                qpend = []  # deferred rope stage2 for the st3 Q projections
                sl3 = slice((NST - 1) * ST, NST * ST)

                def flush_qpend():
                    while qpend:
                        r, ph = qpend.pop(0)
                        swp_t = ppO.tile([P, ST], F32, tag="po", name=f"swp3_{ph}")
                        rope_stage2(r, swp_t[:], cq_t[:, sl3], sq_t[:, sl3],
                                    qh[ph][:, sl3], ropeP, ST, "r", f"q3_{ph}",
                                    dve=True)

                qp3 = {}  # in-flight deferred Q(st3) projection of this unit

                def emit_qproj3_part(h, part):
                    # Q projection of the last s-tile, deferred into phase B
                    # as PE fill work while ACT paces q-tile 0; two half-
                    # contractions so the po-ring slot isn't held too long.
                    if part == 0:
                        qp3["t"] = ppO.tile([P, ST], F32, tag="po", name=f"pq3_{h}")
                    for dc in range(part * DC // 2, (part + 1) * DC // 2):
                        nc.tensor.matmul(
                            qp3["t"][:],
                            lhsT=wq_sb[h][:, dc, :],
                            rhs=xslice(xtq[NST - 1], dc),
                            start=(dc == 0),
                            stop=(dc == DC - 1),
                        )
                    if part == 1:
                        raw_sb = rope_stage1(qp3["t"][:], ropeP, ST, "r", f"q3_{h}",
                                             dve=True)
                        qpend.append((raw_sb, h))

                def flush_av(n):
                    while len(avq) > n:
                        avt, kc, ptt, i, fkv = avq.pop(0)
                        nc.tensor.matmul(
                            avt[:],
                            lhsT=vt[:, kc, fkv * P:(fkv + 1) * P],
                            rhs=ptt[:, i * ST:(i + 1) * ST],
                            start=(kc == 0),
                            stop=(kc == NKC - 1),
                        )

                for qt in range(NST if phases == "all" else 0):
                    qsl = slice(qt * ST, (qt + 1) * ST)
                    att_set = [attp.tile([P, ST], BF16, tag=f"att{h}",
                                         name=f"att{qt}_{h}") for h in range(HL)]
                    for h in range(HL):
                        uid = qt * HL + h
                        kv = h // (HL // KVL)
                        av = ppAv.tile([P, ST], F32, tag="av")
                        u = {"id": uid, "av": av, "att": att_set[h]}
                        pt = [None] * 8

                        run = None  # running sum of pt tiles (DVE, bf16 2x)
                        for kp in range(8):
                            sc_ps = ppSc.tile([P, 2 * ST], F32, tag="scores",
                                              bufs=2)
                            for i in range(2):
                                kc = kp * 2 + i
                                nc.tensor.matmul(
                                    sc_ps[:, i * ST:(i + 1) * ST],
                                    lhsT=kt[:, kv, kc * P:(kc + 1) * P],
                                    rhs=qh[h][:, qsl],
                                    start=True,
                                    stop=True,
                                )
                            if kp == 1:
                                flush_qpend()
                            if qt == 0 and phases == "all" and kp in (3, 5):
                                emit_qproj3_part(h, (kp - 3) // 2)
                            if kp == 2 and prev is not None:
                                emit_den(prev)
                                emit_normalize(prev)
                                prev = None
                            pt[kp] = ptp.tile([P, 2 * ST], BF16, tag=f"pt{kp}",
                                              name=f"pt{uid}_{kp}")
                            nc.scalar.activation(pt[kp][:], sc_ps[:], AF.Exp)
                            for i in range(2):
                                avq.append((av, kp * 2 + i, pt[kp], i, kv))
                            flush_av(4)  # av trails scores by two kp
                            if kp > 0:
                                nxt = treep.tile([P, 2 * ST], BF16,
                                                 tag=f"rs{kp % 2}",
                                                 name=f"rs_{uid}_{kp}")
                                nc.vector.tensor_tensor(
                                    nxt[:], run[:] if kp > 1 else pt[0][:],
                                    pt[kp][:], AL.add
                                )
                                run = nxt
                            # half an o_proj chain of the previous q-tile
                            # every other kp
                            if kp % 2 == 1:
                                pop_ohalf()
                        t4 = treep.tile([P, ST], BF16, tag="t4", name=f"t4_{uid}")
                        nc.vector.tensor_tensor(
                            t4[:], run[:, 0:ST], run[:, ST:2 * ST], AL.add
                        )
                        u["t4"] = t4
                        prev = u
                    for qc in range(ST // P):
                        for ot in range(HID // ST):
                            oq.append((qt, att_set, qc, ot))
                # drain: pending AVs, last unit's den/normalize, last o_proj
                flush_av(0)
                if prev is not None:
                    emit_den(prev)
                    emit_normalize(prev)
                pop_ochain(len(oq))

    nc.compile()
    _CACHE["nc"] = nc
    return nc


def _host_inputs(x, Wq, Wk, Wv, Wo):
    """Build the 8 per-core input maps (numpy only)."""
    bf = ml_dtypes.bfloat16

    # rope tables: row j uses frequency j%64
    inv_ts = ROPE_THETA ** (-2.0 * np.arange(D // 2) / D)
    inv_full = np.concatenate([inv_ts, inv_ts])
    pos = np.arange(S, dtype=np.float64)
    ang = inv_full[:, None] * pos[None, :]
    cos_k = np.cos(ang).astype(bf)
    sin_k = np.sin(ang).astype(bf)
    scale = 1.0 / math.sqrt(D)
    cos_q = (np.cos(ang) * scale).astype(bf)
    sin_q = (np.sin(ang) * scale).astype(bf)

    pmat = np.zeros((P, P), np.float32)  # lhsT: swap[i] = -q[i+64] (i<64), +q[i-64]
    for i in range(64):
        pmat[i + 64, i] = -1.0
        pmat[i, i + 64] = 1.0
    pmat = pmat.astype(bf)
    ones = np.ones((P, 1), bf)

    in_maps = []
    for c in range(8):
        b, hg = c // 2, c % 2
        hsl = slice(hg * HL, (hg + 1) * HL)
        kvsl = slice(hg * KVL, (hg + 1) * KVL)
        xTb = np.ascontiguousarray(
            x[b].T.reshape(DC, P, S).transpose(1, 0, 2)
        ).astype(bf)  # [p, dc, s]
        wq_sw = np.ascontiguousarray(
            Wq[:, hsl, :].reshape(DC, P, HL, P).transpose(2, 1, 0, 3)
        ).astype(bf)  # [h, p, dc, j]
        wk_sw = np.ascontiguousarray(
            Wk[:, kvsl, :].reshape(DC, P, KVJ).transpose(1, 0, 2)
        ).astype(bf)  # [p, dc, j]
        wv_sw = np.ascontiguousarray(
            Wv[:, kvsl, :].reshape(DC, P, KVJ).transpose(1, 0, 2)
        ).astype(bf)
        wo_sw = np.ascontiguousarray(Wo[hsl]).astype(bf)  # [h, j(=d), o]
        in_maps.append(
            {
                "xT": xTb,
                "wq": wq_sw,
                "wk": wk_sw,
                "wv": wv_sw,
                "wo": wo_sw,
                "cos_q": cos_q,
                "sin_q": sin_q,
                "cos_k": cos_k,
                "sin_k": sin_k,
                "pmat": pmat,
                "ones": ones,
            }
        )
    return in_maps


def kernel(x, Wq, Wk, Wv, Wo, _trace=False):
    x, Wq, Wk, Wv, Wo = (np.asarray(a, dtype=np.float32) for a in (x, Wq, Wk, Wv, Wo))
    nc = build_nc()
    in_maps = _host_inputs(x, Wq, Wk, Wv, Wo)
    res = run_bass_kernel_spmd(nc, in_maps, core_ids=list(range(8)), trace=_trace)
    out = np.empty((B, S, HID), np.float32)
    for b in range(B):
        out[b] = res.results[2 * b]["out"]
        out[b] += res.results[2 * b + 1]["out"]
    if _trace:
        kernel.last_results = res
    return out
